# revision 12
# baseline (speedup 1.0000x reference)
"""Trainium2 Bass kernel for nn_Decoder_60627758350737 (GNN message passing).

Sharding: node dim N=2048 split across 8 cores (256 nodes each). All MLP
weights replicated; no collectives.

The reference computes layer_edge_features once from the *initial* node
features, and neighbor_indices is a host-visible input — so the neighbor
gather nf0[idx] is done on the host during marshaling and shipped
feature-major alongside the edge features ([128, 6, 1024] fp16 per chunk:
kt 0-2 edges, kt 3-5 gathered nf0). The device applies [W0e | W0n] as a
768-deep contraction inside h0 — no on-device gather at all.

All message-MLP matmuls run in fp16 (full PE rate + fast weight load;
fp32-HIGH mode is ~1.3x slower per column and blocks FWL). Layout is
feature-major ([feature_part, token]); tokens are processed in 12 chunks
of 1024 (k-major: t = k*256 + n). Per chunk (paired 512-col matmuls so
each weight block loads once):
  h0   = gelu(W0en.T@[edge|nfg] + I.T@xw_b)   (xw_b = x@W0x + b0)
  h1   = gelu(W1.T@h0 + b1)
  hsum = sum of the 4 k-group slices of h1 (vector, fp16)
  agg += W2.T@hsum                            (PSUM accumulate across chunks)
then x1 = LN(x + agg/30 + 1.6*b2); x = mask*LN(x1 + dense(x1)) in fp32.
LN mean/var via ones-vector matmuls and rank-1 outer-product apply.
NOTE: agg's three 256-col regions share PSUM banks; only the first region
per bank may issue start=True (bank-wide pending-zero semantics).
"""
import numpy as np
import concourse.bass as bass
import concourse.bacc as bacc
import concourse.mybir as mybir
from concourse import tile
from concourse.bass_utils import run_bass_kernel_spmd
from contextlib import ExitStack

F32 = mybir.dt.float32
F32R = mybir.dt.float32r
F16 = mybir.dt.float16
AF = mybir.ActivationFunctionType

N, K, NF, L = 2048, 48, 384, 3
NCORES = 8
NLOC = N // NCORES            # 256
T = NLOC * K                  # 12288 tokens (k-major: t = k*256 + n)
CH = 1024                     # chunk (tokens)
NCH = T // CH                 # 12
SCALE = 30.0
EPS = 1e-5

# fp16 weight blob column offsets (per layer, [128, WMC])
O_W0X = 0
O_W0EN = 1152
O_W1 = 3456
O_W2 = 4608
O_DW0 = 5760
O_DW1 = 10368
WMC = 14976
# f32 bias blob columns (per layer, [128, WFC])
O_B0 = 0
O_B1 = 3
O_B2 = 6
O_DB0 = 9
O_DB1 = 21
WFC = 24

_NC_CACHE = {}


def _emit(act=None, layers=L):
    act = AF.Gelu if act is None else act
    nc = bacc.Bacc()
    edge_p = nc.declare_dram_parameter("edge", [NCH, 128, 6, CH], F16, isOutput=False)
    wm_p = nc.declare_dram_parameter("wm", [L, 128, WMC], F16, isOutput=False)
    wf_p = nc.declare_dram_parameter("wf", [L, 128, WFC], F32, isOutput=False)
    ln_p = nc.declare_dram_parameter("lnpk", [L, 1, 1536], F32R, isOutput=False)
    cst_p = nc.declare_dram_parameter("consts", [128, 385], F32R, isOutput=False)
    x0_p = nc.declare_dram_parameter("x0", [128, 3, NLOC], F32R, isOutput=False)
    mask_p = nc.declare_dram_parameter("mask", [1, NLOC], F32, isOutput=False)
    out_p = nc.declare_dram_parameter("out_x", [128, 3, NLOC], F32, isOutput=True)

    with tile.TileContext(nc) as tc, ExitStack() as ctx:
        wpool = ctx.enter_context(tc.tile_pool(name="w", bufs=2))
        epool = ctx.enter_context(tc.tile_pool(name="ep", bufs=2))
        xgpool = ctx.enter_context(tc.tile_pool(name="xgp", bufs=2))
        xg1 = ctx.enter_context(tc.tile_pool(name="xg1", bufs=1))
        hpool = ctx.enter_context(tc.tile_pool(name="hp", bufs=2))
        work1 = ctx.enter_context(tc.tile_pool(name="work1", bufs=1))
        xpool = ctx.enter_context(tc.tile_pool(name="xp", bufs=2))
        small = ctx.enter_context(tc.tile_pool(name="small", bufs=1))
        mm = ctx.enter_context(tc.tile_pool(name="mm", bufs=2, space="PSUM"))
        aggp = ctx.enter_context(tc.tile_pool(name="aggp", bufs=1, space="PSUM"))

        # --- one-time loads ---
        cst = small.tile([128, 385], F32R, tag="cst")
        nc.gpsimd.dma_start(cst[:], cst_p[:])
        maskt = small.tile([1, NLOC], F32, tag="maskt")
        nc.gpsimd.dma_start(maskt[:], mask_p[:])
        x = xpool.tile([128, 3, NLOC], F32R, tag="x")
        nc.gpsimd.dma_start(x[:], x0_p[:])

        ones_col = cst[:, 0:1]            # [128,1] ones (stats lhsT)
        ones_row = cst[0:1, 129:385]      # [1,256] ones

        et0 = epool.tile([128, 6, CH], F16, tag="et", name="et")
        nc.sync.dma_start(et0[:], edge_p[0])

        for l in range(layers):
            wm = wpool.tile([128, WMC], F16, tag="wm")
            (nc.gpsimd if l == 0 else nc.sync).dma_start(
                wm[:, 0:O_W1], wm_p[l][:, 0:O_W1])
            nc.gpsimd.dma_start(wm[:, O_W1:O_DW0], wm_p[l][:, O_W1:O_DW0])
            nc.gpsimd.dma_start(wm[:, O_DW0:WMC], wm_p[l][:, O_DW0:WMC])
            wf = wpool.tile([128, WFC], F32, tag="wf")
            nc.gpsimd.dma_start(wf[:], wf_p[l])
            lnw = wpool.tile([1, 1536], F32R, tag="lnw")
            nc.gpsimd.dma_start(lnw[:], ln_p[l])

            # --- xw_b = x @ W0x + b0  (fp16, feature-major), replicated x4 ---
            xb = xpool.tile([128, 3, NLOC], F16, tag="xb")
            nc.vector.tensor_copy(xb[:], x[:].bitcast(F32))
            xwp = mm.tile([128, 3, 512], F32, tag="mm")
            for mt in range(3):
                for kt in range(3):
                    nc.tensor.matmul(
                        xwp[:, mt, 0:256],
                        wm[:, O_W0X + kt * 384 + mt * 128: O_W0X + kt * 384 + (mt + 1) * 128],
                        xb[:, kt, :],
                        start=(kt == 0), stop=(kt == 2))
            xw4 = work1.tile([128, 3, CH], F16, tag="xw4")
            for mt in range(3):
                nc.scalar.activation(xw4[:, mt, 0:256], xwp[:, mt, 0:256],
                                     AF.Identity,
                                     bias=wf[:, O_B0 + mt:O_B0 + mt + 1])
            for r in range(1, 4):
                nc.vector.tensor_copy(xw4[:, :, r * 256:(r + 1) * 256],
                                      xw4[:, :, 0:256])

            # b2s = b2 * (K/SCALE); db1b staged for broadcast adds
            b2s = small.tile([128, 3, 1], F32, tag="b2s")
            nc.vector.tensor_scalar_mul(
                b2s[:].rearrange("p a b -> p (a b)"),
                wf[:, O_B2:O_B2 + 3], K / SCALE)
            db1b = small.tile([128, 3, 1], F32, tag="db1b")
            nc.vector.tensor_copy(db1b[:].rearrange("p a b -> p (a b)"),
                                  wf[:, O_DB1:O_DB1 + 3])

            # --- k loop: 24 software-pipelined half-iterations of 512
            # tokens. Emission order per step i keeps the PE stream dense:
            # hp(i) MMs run while h0g(i-1) act / h1g(i-2) acts complete, so
            # the PSUM WAR chain never stalls the PE. agg for chunk c is
            # delayed two steps so its hsum input is long done.
            agg = aggp.tile([128, 768], F32, tag="agg")
            NH = 2 * NCH
            et_t = [None] * NCH
            h0g_t = [None] * NH
            h1g_t = [None] * NH
            sh_t = [None] * NH

            def emit_h0(i):
                cc, h = i // 2, i % 2
                if h == 0:
                    if l == 0 and cc == 0:
                        et_t[cc] = et0
                    else:
                        et_t[cc] = epool.tile([128, 6, CH], F16, tag="et",
                                              name="et")
                        nc.sync.dma_start(et_t[cc][:], edge_p[cc])
                et = et_t[cc]
                hp = mm.tile([128, 3, 512], F32, tag="mm", name="hp")
                for mt in range(3):
                    for kt in range(6):
                        off = O_W0EN + kt * 384 + mt * 128
                        nc.tensor.matmul(
                            hp[:, mt, :], wm[:, off:off + 128],
                            et[:, kt, h * 512:(h + 1) * 512],
                            start=(kt == 0), stop=(kt == 5))
                h0pre = hpool.tile([128, 3, 512], F16, tag=f"h0pre{i % 2}",
                                   name="h0pre")
                nc.vector.tensor_add(h0pre[:], hp[:],
                                     xw4[:, :, h * 512:(h + 1) * 512])
                h0g_t[i] = hpool.tile([128, 3, 512], F16, tag=f"h0g{i % 2}",
                                      name="h0g")
                nc.scalar.activation(h0g_t[i][:].rearrange("p a b -> p (a b)"),
                                     h0pre[:].rearrange("p a b -> p (a b)"),
                                     act)

            def emit_h1(i):
                h1p = mm.tile([128, 3, 512], F32, tag="mm", name="h1p")
                for mt in range(3):
                    for kt in range(3):
                        off = O_W1 + kt * 384 + mt * 128
                        nc.tensor.matmul(
                            h1p[:, mt, :], wm[:, off:off + 128],
                            h0g_t[i][:, kt, :],
                            start=(kt == 0), stop=(kt == 2))
                h1g = hpool.tile([128, 3, 512], F16, tag=f"h1g{i % 2}",
                                 name="h1g")
                for mt in range(3):
                    nc.scalar.activation(h1g[:, mt, :], h1p[:, mt, :], act,
                                         bias=wf[:, O_B1 + mt:O_B1 + mt + 1])
                sh_t[i] = xg1.tile([128, 3, 256], F16, tag=f"sh{i % 5}",
                                     name="sh")
                nc.vector.tensor_add(sh_t[i][:], h1g[:, :, 0:256],
                                     h1g[:, :, 256:512])

            def emit_agg(p):
                # p indexes a pair of chunks (8 k-groups pre-summed on DVE).
                # mt=1 shares a PSUM bank with mt=0: start=True pends the
                # whole 2KB zero region, so mt=1 must NOT issue start.
                ha = xg1.tile([128, 3, 256], F16, tag="ha", name="ha")
                nc.vector.tensor_add(ha[:], sh_t[4 * p][:], sh_t[4 * p + 1][:])
                hb = xg1.tile([128, 3, 256], F16, tag="hb", name="hb")
                nc.vector.tensor_add(hb[:], sh_t[4 * p + 2][:],
                                     sh_t[4 * p + 3][:])
                hs = xg1.tile([128, 3, 256], F16, tag="hs", name="hs")
                nc.vector.tensor_add(hs[:], ha[:], hb[:])
                for mt in range(3):
                    for kt in range(3):
                        off = O_W2 + kt * 384 + mt * 128
                        nc.tensor.matmul(
                            agg[:, mt * 256:(mt + 1) * 256],
                            wm[:, off:off + 128], hs[:, kt, :],
                            start=(p == 0 and kt == 0 and mt != 1),
                            stop=(p == NCH // 2 - 1 and kt == 2),
                            skip_group_check=True)

            for i in range(NH + 2):
                if i < NH:
                    emit_h0(i)
                if 1 <= i <= NH:
                    emit_h1(i - 1)
                if i >= 5 and (i - 5) % 4 == 0 and (i - 5) // 4 < NCH // 2:
                    emit_agg((i - 5) // 4)

            # --- x1_pre = x + agg + b2*K/SCALE (W2 is host-scaled by 1/30)
            dum = small.tile([1, 1], F32, tag="dum")
            nc.scalar.activation(dum[:], cst[0:1, 0:1].bitcast(F32), AF.Sqrt)
            t1 = work1.tile([128, 3, 256], F32, tag="t1")
            nc.vector.tensor_add(t1[:], agg[:].rearrange("p (a b) -> p a b", b=256),
                                 b2s[:].to_broadcast([128, 3, 256]))
            x1p = work1.tile([128, 3, 256], F32R, tag="x1p")
            nc.vector.tensor_add(x1p[:], x[:].bitcast(F32), t1[:])

            def layernorm(src, ln_i, masked):
                """src: [128,3,256] F32R tile -> returns new [128,3,256] tile."""
                sq = work1.tile([128, 3, 256], F32R, tag="sq")
                nc.vector.tensor_mul(sq[:], src[:].bitcast(F32),
                                     src[:].bitcast(F32))
                st = mm.tile([128, 3, 512], F32, tag="mm")
                for kt in range(3):
                    nc.tensor.matmul(st[0:1, 0, 0:256], ones_col, src[:, kt, :],
                                     start=(kt == 0), stop=(kt == 2))
                for kt in range(3):
                    nc.tensor.matmul(st[0:1, 0, 256:512], ones_col, sq[:, kt, :],
                                     start=(kt == 0), stop=(kt == 2))
                sm = small.tile([1, 512], F32, tag="sm")
                nc.vector.tensor_scalar_mul(sm[:], st[0:1, 0, :], 1.0 / NF)
                msq = small.tile([1, 256], F32, tag="msq")
                nc.vector.tensor_mul(msq[:], sm[0:1, 0:256], sm[0:1, 0:256])
                var = small.tile([1, 256], F32, tag="var")
                nc.vector.tensor_sub(var[:], sm[0:1, 256:512], msq[:])
                nc.vector.tensor_scalar_add(var[:], var[:], EPS)
                sd = small.tile([1, 256], F32, tag="sd")
                nc.scalar.activation(sd[:], var[:], AF.Sqrt)
                rstd = small.tile([1, 256], F32, tag="rstd")
                nc.vector.reciprocal_approx_fast(rstd[:], sd[:])
                # rv rows (f32r): [0:256]=rstd(*mask), [256:512]=-m*rstd(*mask),
                # [512:768]= ones or mask
                rv = small.tile([1, 768], F32R, tag="rv")
                negm = small.tile([1, 256], F32, tag="negm")
                nc.vector.tensor_scalar_mul(negm[:], sm[0:1, 0:256], -1.0)
                nmr = small.tile([1, 256], F32, tag="nmr")
                nc.vector.tensor_mul(nmr[:], negm[:], rstd[:])
                if masked:
                    nc.vector.tensor_mul(rv[0:1, 0:256], rstd[:], maskt[:])
                    nc.vector.tensor_mul(rv[0:1, 256:512], nmr[:], maskt[:])
                    nc.vector.tensor_copy(rv[0:1, 512:768], maskt[:])
                else:
                    nc.vector.tensor_copy(rv[0:1, 0:256], rstd[:])
                    nc.vector.tensor_copy(rv[0:1, 256:512], nmr[:])
                    nc.vector.tensor_copy(rv[0:1, 512:768], ones_row.bitcast(F32))
                stp = mm.tile([128, 3, 512], F32, tag="mm")
                for mt in range(3):
                    woff = ln_i * 384 + mt * 128
                    nc.tensor.matmul(stp[:, mt, 0:256], lnw[0:1, woff:woff + 128],
                                     rv[0:1, 0:256], start=True, stop=True)
                    nc.tensor.matmul(stp[:, mt, 256:512],
                                     lnw[0:1, 768 + woff:768 + woff + 128],
                                     rv[0:1, 512:768], start=True, stop=False)
                    nc.tensor.matmul(stp[:, mt, 256:512], lnw[0:1, woff:woff + 128],
                                     rv[0:1, 256:512], start=False, stop=True)
                outt = xpool.tile([128, 3, 256], F32R, tag="lnout")
                for mt in range(3):
                    nc.vector.tensor_mul(outt[:, mt, :], src[:, mt, :].bitcast(F32),
                                         stp[:, mt, 0:256])
                    nc.vector.tensor_add(outt[:, mt, :], outt[:, mt, :].bitcast(F32),
                                         stp[:, mt, 256:512])
                return outt

            x1 = layernorm(x1p, 0, masked=False)
            x1b = xpool.tile([128, 3, 256], F16, tag="x1b")
            nc.vector.tensor_copy(x1b[:], x1[:].bitcast(F32))

            # --- dense MLP: d0 = gelu(x1 @ dw0 + db0); d1 = d0 @ dw1 + db1 ---
            d0g = work1.tile([128, 12, 256], F16, tag="d0g")
            for half in range(2):
                dp = mm.tile([128, 3, 512], F32, tag="mm")
                for m6 in range(6):
                    mt = half * 6 + m6
                    reg = dp[:, m6 // 2, (m6 % 2) * 256:(m6 % 2) * 256 + 256]
                    for kt in range(3):
                        nc.tensor.matmul(
                            reg,
                            wm[:, O_DW0 + kt * 1536 + mt * 128: O_DW0 + kt * 1536 + (mt + 1) * 128],
                            x1b[:, kt, :],
                            start=(kt == 0), stop=(kt == 2))
                    nc.scalar.activation(d0g[:, mt, :], reg, act,
                                         bias=wf[:, O_DB0 + mt:O_DB0 + mt + 1])
            d1p = mm.tile([128, 3, 512], F32, tag="mm")
            for mt in range(3):
                for kt in range(12):
                    nc.tensor.matmul(
                        d1p[:, mt, 0:256],
                        wm[:, O_DW1 + kt * 384 + mt * 128: O_DW1 + kt * 384 + (mt + 1) * 128],
                        d0g[:, kt, :],
                        start=(kt == 0), stop=(kt == 11))
            dum2 = small.tile([1, 1], F32, tag="dum2")
            nc.scalar.activation(dum2[:], cst[0:1, 0:1].bitcast(F32), AF.Sqrt)
            t2 = work1.tile([128, 3, 256], F32, tag="t2")
            nc.vector.tensor_add(t2[:], d1p[:, :, 0:256],
                                 db1b[:].to_broadcast([128, 3, 256]))
            x2p = work1.tile([128, 3, 256], F32R, tag="x2p")
            nc.vector.tensor_add(x2p[:], x1[:].bitcast(F32), t2[:])

            x = layernorm(x2p, 1, masked=True)

        nc.sync.dma_start(out_p[:], x[:].bitcast(F32))

    nc.finalize()
    return nc


def _get_nc():
    if "nc" not in _NC_CACHE:
        _NC_CACHE["nc"] = _emit()
    return _NC_CACHE["nc"]


def _fm(w):
    """[in, out] fp32 -> [128, n_kt*out] (feature-major lhsT blob columns)."""
    i, o = w.shape
    return np.ascontiguousarray(
        w.reshape(i // 128, 128, o).transpose(1, 0, 2).reshape(128, -1))


def _marshal(inputs):
    nf = np.asarray(inputs["node_features"], np.float32)
    ef = np.asarray(inputs["edge_features"], np.float32)
    idx = np.asarray(inputs["neighbor_indices"])
    mask = np.asarray(inputs["mask"], np.float32)

    # replicated tensors
    wm = np.empty((L, 128, WMC), np.float16)
    wf = np.empty((L, 128, WFC), np.float32)
    lnpk = np.empty((L, 1, 1536), np.float32)
    for l in range(L):
        w0 = np.asarray(inputs["msg_w0"], np.float32)[l]
        cols = [
            _fm(w0[0:384]), _fm(w0[384:768]), _fm(w0[1152:1536]),
            _fm(np.asarray(inputs["msg_w1"], np.float32)[l]),
            _fm(np.asarray(inputs["msg_w2"], np.float32)[l] / SCALE),
            _fm(np.asarray(inputs["dense_w0"], np.float32)[l]),
            _fm(np.asarray(inputs["dense_w1"], np.float32)[l]),
        ]
        wm[l] = np.concatenate(cols, axis=1).astype(np.float16)
        fcols = [
            np.asarray(inputs["msg_b0"], np.float32)[l].reshape(3, 128).T,
            np.asarray(inputs["msg_b1"], np.float32)[l].reshape(3, 128).T,
            np.asarray(inputs["msg_b2"], np.float32)[l].reshape(3, 128).T,
            np.asarray(inputs["dense_b0"], np.float32)[l].reshape(12, 128).T,
            np.asarray(inputs["dense_b1"], np.float32)[l].reshape(3, 128).T,
        ]
        wf[l] = np.concatenate(fcols, axis=1)
        lnpk[l, 0] = np.concatenate([
            np.asarray(inputs["ln1_w"], np.float32)[l],
            np.asarray(inputs["ln2_w"], np.float32)[l],
            np.asarray(inputs["ln1_b"], np.float32)[l],
            np.asarray(inputs["ln2_b"], np.float32)[l]])
    consts = np.zeros((128, 385), np.float32)
    consts[:, 0] = 1.0
    consts[:, 1:129] = np.eye(128, dtype=np.float32)
    consts[0, 129:385] = 1.0

    nf16 = nf.astype(np.float16)
    in_maps = []
    for c in range(NCORES):
        lo = slice(c * NLOC, (c + 1) * NLOC)
        el = ef[lo]                                        # [256,48,384]
        E = el.transpose(1, 0, 2).reshape(T, 384).astype(np.float16)  # k-major
        idx_k = np.ascontiguousarray(idx[lo].T).reshape(T)     # k-major values
        nfg = nf16[idx_k]                                  # [T,384] host gather
        comb = np.concatenate([E, nfg], axis=1)            # [T,768]
        edge = np.ascontiguousarray(
            comb.reshape(NCH, CH, 6, 128).transpose(0, 3, 2, 1))
        x0 = np.ascontiguousarray(
            nf[lo].reshape(NLOC, 3, 128).transpose(2, 1, 0))   # [128,3,256]
        in_maps.append(dict(
            edge=edge, wm=wm, wf=wf, lnpk=lnpk,
            consts=consts, x0=x0,
            mask=np.ascontiguousarray(mask[lo])[None, :]))
    return in_maps


def _unshard(results):
    out = np.empty((N, NF), np.float32)
    for c in range(NCORES):
        xfm = results[c]["out_x"]                          # [128,3,256]
        out[c * NLOC:(c + 1) * NLOC] = xfm.transpose(2, 1, 0).reshape(NLOC, NF)
    return out


def kernel(**inputs):
    nc = _get_nc()
    in_maps = _marshal(inputs)
    res = run_bass_kernel_spmd(nc, in_maps, list(range(NCORES)), trace=False)
    return _unshard(res.results)


# revision 13
# speedup vs baseline: 1.1804x; 1.1804x over previous
"""Trainium2 Bass kernel for nn_Decoder_60627758350737 (GNN message passing).

Sharding: node dim N=2048 split across 8 cores (256 nodes each). All MLP
weights replicated; no collectives.

The reference computes layer_edge_features once from the *initial* node
features, and neighbor_indices is a host-visible input — so the neighbor
gather nf0[idx] is done on the host during marshaling and shipped
feature-major alongside the edge features ([128, 6, 1024] fp16 per chunk:
kt 0-2 edges, kt 3-5 gathered nf0). The device applies [W0e | W0n] as a
768-deep contraction inside h0 — no on-device gather at all.

All message-MLP matmuls run in fp16 (full PE rate + fast weight load;
fp32-HIGH mode is ~1.3x slower per column and blocks FWL). Layout is
feature-major ([feature_part, token]); tokens are processed in 12 chunks
of 1024 (k-major: t = k*256 + n). Per chunk (paired 512-col matmuls so
each weight block loads once):
  h0   = gelu(W0en.T@[edge|nfg] + I.T@xw_b)   (xw_b = x@W0x + b0)
  h1   = gelu(W1.T@h0 + b1)
  hsum = sum of the 4 k-group slices of h1 (vector, fp16)
  agg += W2.T@hsum                            (PSUM accumulate across chunks)
then x1 = LN(x + agg/30 + 1.6*b2); x = mask*LN(x1 + dense(x1)) in fp32.
LN mean/var via ones-vector matmuls and rank-1 outer-product apply.
NOTE: agg's three 256-col regions share PSUM banks; only the first region
per bank may issue start=True (bank-wide pending-zero semantics).
"""
import numpy as np
import concourse.bass as bass
import concourse.bacc as bacc
import concourse.mybir as mybir
from concourse import tile
from concourse.bass_utils import run_bass_kernel_spmd
from contextlib import ExitStack

F32 = mybir.dt.float32
F32R = mybir.dt.float32r
F16 = mybir.dt.float16
AF = mybir.ActivationFunctionType

N, K, NF, L = 2048, 48, 384, 3
NCORES = 8
NLOC = N // NCORES            # 256
T = NLOC * K                  # 12288 tokens (k-major: t = k*256 + n)
CH = 1024                     # chunk (tokens)
NCH = T // CH                 # 12
SCALE = 30.0
EPS = 1e-5

# fp16 weight blob column offsets (per layer, [128, WMC])
O_W0X = 0
O_W0EN = 1152
O_W1 = 3456
O_W2 = 4608
O_DW0 = 5760
O_DW1 = 10368
WMC = 14976
# f32 bias blob columns (per layer, [128, WFC])
O_B0 = 0
O_B1 = 3
O_B2 = 6
O_DB0 = 9
O_DB1 = 21
WFC = 24

_NC_CACHE = {}


def _emit(act=None, layers=L):
    act = AF.Gelu if act is None else act
    nc = bacc.Bacc()
    edge_p = nc.declare_dram_parameter("edge", [NCH, 128, 6, CH], F16, isOutput=False)
    wm_p = nc.declare_dram_parameter("wm", [L, 128, WMC], F16, isOutput=False)
    wf_p = nc.declare_dram_parameter("wf", [L, 128, WFC], F32, isOutput=False)
    ln_p = nc.declare_dram_parameter("lnpk", [L, 1, 1536], F32R, isOutput=False)
    cst_p = nc.declare_dram_parameter("consts", [128, 385], F32R, isOutput=False)
    x0_p = nc.declare_dram_parameter("x0", [128, 3, NLOC], F32R, isOutput=False)
    mask_p = nc.declare_dram_parameter("mask", [1, NLOC], F32, isOutput=False)
    out_p = nc.declare_dram_parameter("out_x", [128, 3, NLOC], F32, isOutput=True)

    with tile.TileContext(nc) as tc, ExitStack() as ctx:
        wpool = ctx.enter_context(tc.tile_pool(name="w", bufs=2))
        epool = ctx.enter_context(tc.tile_pool(name="ep", bufs=2))
        xgpool = ctx.enter_context(tc.tile_pool(name="xgp", bufs=2))
        xg1 = ctx.enter_context(tc.tile_pool(name="xg1", bufs=1))
        hpool = ctx.enter_context(tc.tile_pool(name="hp", bufs=2))
        work1 = ctx.enter_context(tc.tile_pool(name="work1", bufs=1))
        xpool = ctx.enter_context(tc.tile_pool(name="xp", bufs=2))
        small = ctx.enter_context(tc.tile_pool(name="small", bufs=1))
        mm = ctx.enter_context(tc.tile_pool(name="mm", bufs=2, space="PSUM"))
        aggp = ctx.enter_context(tc.tile_pool(name="aggp", bufs=1, space="PSUM"))

        # --- one-time loads ---
        cst = small.tile([128, 385], F32R, tag="cst")
        nc.gpsimd.dma_start(cst[:], cst_p[:])
        maskt = small.tile([1, NLOC], F32, tag="maskt")
        nc.gpsimd.dma_start(maskt[:], mask_p[:])
        x = xpool.tile([128, 3, NLOC], F32R, tag="x")
        nc.gpsimd.dma_start(x[:], x0_p[:])

        ones_col = cst[:, 0:1]            # [128,1] ones (stats lhsT)
        ones_row = cst[0:1, 129:385]      # [1,256] ones

        et0 = epool.tile([128, 6, CH], F16, tag="et", name="et")
        nc.sync.dma_start(et0[:], edge_p[0])

        for l in range(layers):
            wm = wpool.tile([128, WMC], F16, tag="wm")
            nc.sync.dma_start(wm[:, 0:O_W1], wm_p[l][:, 0:O_W1])
            nc.gpsimd.dma_start(wm[:, O_W1:O_DW0], wm_p[l][:, O_W1:O_DW0])
            nc.gpsimd.dma_start(wm[:, O_DW0:WMC], wm_p[l][:, O_DW0:WMC])
            wf = wpool.tile([128, WFC], F32, tag="wf")
            nc.gpsimd.dma_start(wf[:], wf_p[l])
            lnw = wpool.tile([1, 1536], F32R, tag="lnw")
            nc.gpsimd.dma_start(lnw[:], ln_p[l])

            # --- xw_b = x @ W0x + b0  (fp16, feature-major), replicated x4 ---
            xb = xpool.tile([128, 3, NLOC], F16, tag="xb")
            nc.vector.tensor_copy(xb[:], x[:].bitcast(F32))
            xwp = mm.tile([128, 3, 512], F32, tag="mm")
            for mt in range(3):
                for kt in range(3):
                    nc.tensor.matmul(
                        xwp[:, mt, 0:256],
                        wm[:, O_W0X + kt * 384 + mt * 128: O_W0X + kt * 384 + (mt + 1) * 128],
                        xb[:, kt, :],
                        start=(kt == 0), stop=(kt == 2))
            xw4 = work1.tile([128, 3, CH], F16, tag="xw4")
            for mt in range(3):
                nc.scalar.activation(xw4[:, mt, 0:256], xwp[:, mt, 0:256],
                                     AF.Identity,
                                     bias=wf[:, O_B0 + mt:O_B0 + mt + 1])
            for r in range(1, 4):
                nc.vector.tensor_copy(xw4[:, :, r * 256:(r + 1) * 256],
                                      xw4[:, :, 0:256])

            # b2s = b2 * (K/SCALE)
            b2s = small.tile([128, 3], F32, tag="b2s")
            nc.vector.tensor_scalar_mul(b2s[:], wf[:, O_B2:O_B2 + 3], K / SCALE)

            # --- k loop: 24 software-pipelined half-iterations of 512
            # tokens. Emission order per step i keeps the PE stream dense:
            # hp(i) MMs run while h0g(i-1) act / h1g(i-2) acts complete, so
            # the PSUM WAR chain never stalls the PE. agg for chunk c is
            # delayed two steps so its hsum input is long done.
            agg = aggp.tile([128, 768], F32, tag="agg")
            NH = 2 * NCH
            et_t = [None] * NCH
            h0g_t = [None] * NH
            h1g_t = [None] * NH
            sh_t = [None] * NH

            def emit_h0(i):
                cc, h = i // 2, i % 2
                if h == 0:
                    if l == 0 and cc == 0:
                        et_t[cc] = et0
                    else:
                        et_t[cc] = epool.tile([128, 6, CH], F16, tag="et",
                                              name="et")
                        nc.sync.dma_start(et_t[cc][:], edge_p[cc])
                et = et_t[cc]
                hp = mm.tile([128, 3, 512], F32, tag="mm", name="hp")
                for mt in range(3):
                    for kt in range(6):
                        off = O_W0EN + kt * 384 + mt * 128
                        nc.tensor.matmul(
                            hp[:, mt, :], wm[:, off:off + 128],
                            et[:, kt, h * 512:(h + 1) * 512],
                            start=(kt == 0), stop=(kt == 5))
                h0pre = hpool.tile([128, 3, 512], F16, tag=f"h0pre{i % 2}",
                                   name="h0pre")
                nc.vector.tensor_add(h0pre[:], hp[:],
                                     xw4[:, :, h * 512:(h + 1) * 512])
                h0g_t[i] = hpool.tile([128, 3, 512], F16, tag=f"h0g{i % 2}",
                                      name="h0g")
                nc.scalar.activation(h0g_t[i][:].rearrange("p a b -> p (a b)"),
                                     h0pre[:].rearrange("p a b -> p (a b)"),
                                     act)

            def emit_h1(i):
                h1p = mm.tile([128, 3, 512], F32, tag="mm", name="h1p")
                for mt in range(3):
                    for kt in range(3):
                        off = O_W1 + kt * 384 + mt * 128
                        nc.tensor.matmul(
                            h1p[:, mt, :], wm[:, off:off + 128],
                            h0g_t[i][:, kt, :],
                            start=(kt == 0), stop=(kt == 2))
                h1g = hpool.tile([128, 3, 512], F16, tag=f"h1g{i % 2}",
                                 name="h1g")
                for mt in range(3):
                    nc.scalar.activation(h1g[:, mt, :], h1p[:, mt, :], act,
                                         bias=wf[:, O_B1 + mt:O_B1 + mt + 1])
                sh_t[i] = xg1.tile([128, 3, 256], F16, tag=f"sh{i % 5}",
                                     name="sh")
                nc.vector.tensor_add(sh_t[i][:], h1g[:, :, 0:256],
                                     h1g[:, :, 256:512])

            def emit_agg(p):
                # p indexes a pair of chunks (8 k-groups pre-summed on DVE).
                # mt=1 shares a PSUM bank with mt=0: start=True pends the
                # whole 2KB zero region, so mt=1 must NOT issue start.
                ha = xg1.tile([128, 3, 256], F16, tag="ha", name="ha")
                nc.vector.tensor_add(ha[:], sh_t[4 * p][:], sh_t[4 * p + 1][:])
                hb = xg1.tile([128, 3, 256], F16, tag="hb", name="hb")
                nc.vector.tensor_add(hb[:], sh_t[4 * p + 2][:],
                                     sh_t[4 * p + 3][:])
                hs = xg1.tile([128, 3, 256], F16, tag="hs", name="hs")
                nc.vector.tensor_add(hs[:], ha[:], hb[:])
                for mt in range(3):
                    for kt in range(3):
                        off = O_W2 + kt * 384 + mt * 128
                        nc.tensor.matmul(
                            agg[:, mt * 256:(mt + 1) * 256],
                            wm[:, off:off + 128], hs[:, kt, :],
                            start=(p == 0 and kt == 0 and mt != 1),
                            stop=(p == NCH // 2 - 1 and kt == 2),
                            skip_group_check=True)

            for i in range(NH + 2):
                if i < NH:
                    emit_h0(i)
                if 1 <= i <= NH:
                    emit_h1(i - 1)
                if i >= 5 and (i - 5) % 4 == 0 and (i - 5) // 4 < NCH // 2:
                    emit_agg((i - 5) // 4)

            # --- x1_pre = x + agg + b2*K/SCALE (W2 is host-scaled by 1/30)
            dum = small.tile([1, 1], F32, tag="dum")
            nc.scalar.activation(dum[:], cst[0:1, 0:1].bitcast(F32), AF.Sqrt)
            t1 = work1.tile([128, 3, 256], F32, tag="t1")
            for mt in range(3):
                nc.scalar.activation(t1[:, mt, :], agg[:, mt * 256:(mt + 1) * 256],
                                     AF.Identity, bias=b2s[:, mt:mt + 1])
            x1p = work1.tile([128, 3, 256], F32R, tag="x1p")
            nc.vector.tensor_add(x1p[:], x[:].bitcast(F32), t1[:])

            def layernorm(src, ln_i, masked):
                """src: [128,3,256] F32R tile -> returns new [128,3,256] tile."""
                sq = work1.tile([128, 3, 256], F32R, tag="sq")
                nc.vector.tensor_mul(sq[:], src[:].bitcast(F32),
                                     src[:].bitcast(F32))
                st = mm.tile([128, 3, 512], F32, tag="mm")
                for kt in range(3):
                    nc.tensor.matmul(st[0:1, 0, 0:256], ones_col, src[:, kt, :],
                                     start=(kt == 0), stop=(kt == 2))
                for kt in range(3):
                    nc.tensor.matmul(st[0:1, 0, 256:512], ones_col, sq[:, kt, :],
                                     start=(kt == 0), stop=(kt == 2))
                sm = small.tile([1, 512], F32, tag="sm")
                nc.vector.tensor_scalar_mul(sm[:], st[0:1, 0, :], 1.0 / NF)
                msq = small.tile([1, 256], F32, tag="msq")
                nc.vector.tensor_mul(msq[:], sm[0:1, 0:256], sm[0:1, 0:256])
                var = small.tile([1, 256], F32, tag="var")
                nc.vector.tensor_sub(var[:], sm[0:1, 256:512], msq[:])
                nc.vector.tensor_scalar_add(var[:], var[:], EPS)
                sd = small.tile([1, 256], F32, tag="sd")
                nc.scalar.activation(sd[:], var[:], AF.Sqrt)
                rstd = small.tile([1, 256], F32, tag="rstd")
                nc.vector.reciprocal_approx_fast(rstd[:], sd[:])
                # rv rows (f32r): [0:256]=rstd(*mask), [256:512]=-m*rstd(*mask),
                # [512:768]= ones or mask
                rv = small.tile([1, 768], F32R, tag="rv")
                negm = small.tile([1, 256], F32, tag="negm")
                nc.vector.tensor_scalar_mul(negm[:], sm[0:1, 0:256], -1.0)
                nmr = small.tile([1, 256], F32, tag="nmr")
                nc.vector.tensor_mul(nmr[:], negm[:], rstd[:])
                if masked:
                    nc.vector.tensor_mul(rv[0:1, 0:256], rstd[:], maskt[:])
                    nc.vector.tensor_mul(rv[0:1, 256:512], nmr[:], maskt[:])
                    nc.vector.tensor_copy(rv[0:1, 512:768], maskt[:])
                else:
                    nc.vector.tensor_copy(rv[0:1, 0:256], rstd[:])
                    nc.vector.tensor_copy(rv[0:1, 256:512], nmr[:])
                    nc.vector.tensor_copy(rv[0:1, 512:768], ones_row.bitcast(F32))
                stp = mm.tile([128, 3, 512], F32, tag="mm")
                for mt in range(3):
                    woff = ln_i * 384 + mt * 128
                    nc.tensor.matmul(stp[:, mt, 0:256], lnw[0:1, woff:woff + 128],
                                     rv[0:1, 0:256], start=True, stop=True)
                    nc.tensor.matmul(stp[:, mt, 256:512],
                                     lnw[0:1, 768 + woff:768 + woff + 128],
                                     rv[0:1, 512:768], start=True, stop=False)
                    nc.tensor.matmul(stp[:, mt, 256:512], lnw[0:1, woff:woff + 128],
                                     rv[0:1, 256:512], start=False, stop=True)
                outt = xpool.tile([128, 3, 256], F32R, tag="lnout")
                for mt in range(3):
                    nc.vector.tensor_mul(outt[:, mt, :], src[:, mt, :].bitcast(F32),
                                         stp[:, mt, 0:256])
                    nc.vector.tensor_add(outt[:, mt, :], outt[:, mt, :].bitcast(F32),
                                         stp[:, mt, 256:512])
                return outt

            x1 = layernorm(x1p, 0, masked=False)
            x1b = xpool.tile([128, 3, 256], F16, tag="x1b")
            nc.vector.tensor_copy(x1b[:], x1[:].bitcast(F32))

            # --- dense MLP: d0 = gelu(x1 @ dw0 + db0); d1 = d0 @ dw1 + db1 ---
            d0g = work1.tile([128, 12, 256], F16, tag="d0g")
            for half in range(2):
                dp = mm.tile([128, 3, 512], F32, tag="mm")
                for m6 in range(6):
                    mt = half * 6 + m6
                    reg = dp[:, m6 // 2, (m6 % 2) * 256:(m6 % 2) * 256 + 256]
                    for kt in range(3):
                        nc.tensor.matmul(
                            reg,
                            wm[:, O_DW0 + kt * 1536 + mt * 128: O_DW0 + kt * 1536 + (mt + 1) * 128],
                            x1b[:, kt, :],
                            start=(kt == 0), stop=(kt == 2))
                    nc.scalar.activation(d0g[:, mt, :], reg, act,
                                         bias=wf[:, O_DB0 + mt:O_DB0 + mt + 1])
            d1p = mm.tile([128, 3, 512], F32, tag="mm")
            for mt in range(3):
                for kt in range(12):
                    nc.tensor.matmul(
                        d1p[:, mt, 0:256],
                        wm[:, O_DW1 + kt * 384 + mt * 128: O_DW1 + kt * 384 + (mt + 1) * 128],
                        d0g[:, kt, :],
                        start=(kt == 0), stop=(kt == 11))
            dum2 = small.tile([1, 1], F32, tag="dum2")
            nc.scalar.activation(dum2[:], cst[0:1, 0:1].bitcast(F32), AF.Sqrt)
            t2 = work1.tile([128, 3, 256], F32, tag="t2")
            for mt in range(3):
                nc.scalar.activation(t2[:, mt, :], d1p[:, mt, 0:256], AF.Identity,
                                     bias=wf[:, O_DB1 + mt:O_DB1 + mt + 1])
            x2p = work1.tile([128, 3, 256], F32R, tag="x2p")
            nc.vector.tensor_add(x2p[:], x1[:].bitcast(F32), t2[:])

            x = layernorm(x2p, 1, masked=True)

        nc.sync.dma_start(out_p[:], x[:].bitcast(F32))

    nc.finalize()
    return nc


def _get_nc():
    if "nc" not in _NC_CACHE:
        _NC_CACHE["nc"] = _emit()
    return _NC_CACHE["nc"]


def _fm(w):
    """[in, out] fp32 -> [128, n_kt*out] (feature-major lhsT blob columns)."""
    i, o = w.shape
    return np.ascontiguousarray(
        w.reshape(i // 128, 128, o).transpose(1, 0, 2).reshape(128, -1))


def _marshal(inputs):
    nf = np.asarray(inputs["node_features"], np.float32)
    ef = np.asarray(inputs["edge_features"], np.float32)
    idx = np.asarray(inputs["neighbor_indices"])
    mask = np.asarray(inputs["mask"], np.float32)

    # replicated tensors
    wm = np.empty((L, 128, WMC), np.float16)
    wf = np.empty((L, 128, WFC), np.float32)
    lnpk = np.empty((L, 1, 1536), np.float32)
    for l in range(L):
        w0 = np.asarray(inputs["msg_w0"], np.float32)[l]
        cols = [
            _fm(w0[0:384]), _fm(w0[384:768]), _fm(w0[1152:1536]),
            _fm(np.asarray(inputs["msg_w1"], np.float32)[l]),
            _fm(np.asarray(inputs["msg_w2"], np.float32)[l] / SCALE),
            _fm(np.asarray(inputs["dense_w0"], np.float32)[l]),
            _fm(np.asarray(inputs["dense_w1"], np.float32)[l]),
        ]
        wm[l] = np.concatenate(cols, axis=1).astype(np.float16)
        fcols = [
            np.asarray(inputs["msg_b0"], np.float32)[l].reshape(3, 128).T,
            np.asarray(inputs["msg_b1"], np.float32)[l].reshape(3, 128).T,
            np.asarray(inputs["msg_b2"], np.float32)[l].reshape(3, 128).T,
            np.asarray(inputs["dense_b0"], np.float32)[l].reshape(12, 128).T,
            np.asarray(inputs["dense_b1"], np.float32)[l].reshape(3, 128).T,
        ]
        wf[l] = np.concatenate(fcols, axis=1)
        lnpk[l, 0] = np.concatenate([
            np.asarray(inputs["ln1_w"], np.float32)[l],
            np.asarray(inputs["ln2_w"], np.float32)[l],
            np.asarray(inputs["ln1_b"], np.float32)[l],
            np.asarray(inputs["ln2_b"], np.float32)[l]])
    consts = np.zeros((128, 385), np.float32)
    consts[:, 0] = 1.0
    consts[:, 1:129] = np.eye(128, dtype=np.float32)
    consts[0, 129:385] = 1.0

    nf16 = nf.astype(np.float16)
    in_maps = []
    for c in range(NCORES):
        lo = slice(c * NLOC, (c + 1) * NLOC)
        el = ef[lo]                                        # [256,48,384]
        E = el.transpose(1, 0, 2).reshape(T, 384).astype(np.float16)  # k-major
        idx_k = np.ascontiguousarray(idx[lo].T).reshape(T)     # k-major values
        nfg = nf16[idx_k]                                  # [T,384] host gather
        comb = np.concatenate([E, nfg], axis=1)            # [T,768]
        edge = np.ascontiguousarray(
            comb.reshape(NCH, CH, 6, 128).transpose(0, 3, 2, 1))
        x0 = np.ascontiguousarray(
            nf[lo].reshape(NLOC, 3, 128).transpose(2, 1, 0))   # [128,3,256]
        in_maps.append(dict(
            edge=edge, wm=wm, wf=wf, lnpk=lnpk,
            consts=consts, x0=x0,
            mask=np.ascontiguousarray(mask[lo])[None, :]))
    return in_maps


def _unshard(results):
    out = np.empty((N, NF), np.float32)
    for c in range(NCORES):
        xfm = results[c]["out_x"]                          # [128,3,256]
        out[c * NLOC:(c + 1) * NLOC] = xfm.transpose(2, 1, 0).reshape(NLOC, NF)
    return out


def kernel(**inputs):
    nc = _get_nc()
    in_maps = _marshal(inputs)
    res = run_bass_kernel_spmd(nc, in_maps, list(range(NCORES)), trace=False)
    return _unshard(res.results)


# revision 16
# speedup vs baseline: 1.1857x; 1.0045x over previous
"""Trainium2 Bass kernel for nn_Decoder_60627758350737 (GNN message passing).

Sharding: node dim N=2048 split across 8 cores (256 nodes each). All MLP
weights replicated; no collectives.

The reference computes layer_edge_features once from the *initial* node
features, and neighbor_indices is a host-visible input — so the neighbor
gather nf0[idx] is done on the host during marshaling and shipped
feature-major alongside the edge features ([128, 6, 1024] fp16 per chunk:
kt 0-2 edges, kt 3-5 gathered nf0). The device applies [W0e | W0n] as a
768-deep contraction inside h0 — no on-device gather at all.

All message-MLP matmuls run in fp16 (full PE rate + fast weight load;
fp32-HIGH mode is ~1.3x slower per column and blocks FWL). Layout is
feature-major ([feature_part, token]); tokens are processed in 12 chunks
of 1024 (k-major: t = k*256 + n). Per chunk (paired 512-col matmuls so
each weight block loads once):
  h0   = gelu(W0en.T@[edge|nfg] + I.T@xw_b)   (xw_b = x@W0x + b0)
  h1   = gelu(W1.T@h0 + b1)
  hsum = sum of the 4 k-group slices of h1 (vector, fp16)
  agg += W2.T@hsum                            (PSUM accumulate across chunks)
then x1 = LN(x + agg/30 + 1.6*b2); x = mask*LN(x1 + dense(x1)) in fp32.
LN mean/var via ones-vector matmuls and rank-1 outer-product apply.
NOTE: agg's three 256-col regions share PSUM banks; only the first region
per bank may issue start=True (bank-wide pending-zero semantics).
"""
import numpy as np
import concourse.bass as bass
import concourse.bacc as bacc
import concourse.mybir as mybir
from concourse import tile
from concourse.bass_utils import run_bass_kernel_spmd
from contextlib import ExitStack

F32 = mybir.dt.float32
F32R = mybir.dt.float32r
F16 = mybir.dt.float16
AF = mybir.ActivationFunctionType

N, K, NF, L = 2048, 48, 384, 3
NCORES = 8
NLOC = N // NCORES            # 256
T = NLOC * K                  # 12288 tokens (k-major: t = k*256 + n)
CH = 1024                     # chunk (tokens)
NCH = T // CH                 # 12
SCALE = 30.0
EPS = 1e-5

# fp16 weight blob column offsets (per layer, [128, WMC])
O_W0X = 0
O_W0EN = 1152
O_W1 = 3456
O_W2 = 4608
O_DW0 = 5760
O_DW1 = 10368
WMC = 14976
# f32 bias blob columns (per layer, [128, WFC])
O_B0 = 0
O_B1 = 3
O_B2 = 6
O_DB0 = 9
O_DB1 = 21
WFC = 24

_NC_CACHE = {}


def _emit(act=None, layers=L):
    act = AF.Gelu if act is None else act
    nc = bacc.Bacc()
    edge_p = nc.declare_dram_parameter("edge", [NCH, 128, 6, CH], F16, isOutput=False)
    wm_p = nc.declare_dram_parameter("wm", [L, 128, WMC], F16, isOutput=False)
    wf_p = nc.declare_dram_parameter("wf", [L, 128, WFC], F32, isOutput=False)
    ln_p = nc.declare_dram_parameter("lnpk", [L, 1, 1536], F32R, isOutput=False)
    cst_p = nc.declare_dram_parameter("consts", [128, 386], F32R, isOutput=False)
    x0_p = nc.declare_dram_parameter("x0", [128, 3, NLOC], F32R, isOutput=False)
    mask_p = nc.declare_dram_parameter("mask", [1, NLOC], F32, isOutput=False)
    out_p = nc.declare_dram_parameter("out_x", [128, 3, NLOC], F32, isOutput=True)

    with tile.TileContext(nc) as tc, ExitStack() as ctx:
        wpool = ctx.enter_context(tc.tile_pool(name="w", bufs=2))
        epool = ctx.enter_context(tc.tile_pool(name="ep", bufs=2))
        xgpool = ctx.enter_context(tc.tile_pool(name="xgp", bufs=2))
        xg1 = ctx.enter_context(tc.tile_pool(name="xg1", bufs=1))
        hpool = ctx.enter_context(tc.tile_pool(name="hp", bufs=2))
        work1 = ctx.enter_context(tc.tile_pool(name="work1", bufs=1))
        xpool = ctx.enter_context(tc.tile_pool(name="xp", bufs=2))
        small = ctx.enter_context(tc.tile_pool(name="small", bufs=1))
        mm = ctx.enter_context(tc.tile_pool(name="mm", bufs=2, space="PSUM"))
        aggp = ctx.enter_context(tc.tile_pool(name="aggp", bufs=1, space="PSUM"))

        # --- one-time loads ---
        cst = small.tile([128, 386], F32R, tag="cst")
        nc.gpsimd.dma_start(cst[:], cst_p[:])
        maskt = small.tile([1, NLOC], F32, tag="maskt")
        nc.gpsimd.dma_start(maskt[:], mask_p[:])
        x = xpool.tile([128, 3, NLOC], F32R, tag="x")
        nc.gpsimd.dma_start(x[:], x0_p[:])

        ones_col = cst[:, 0:1]            # [128,1] 1/NF (stats lhsT -> means)
        ones_row = cst[0:1, 129:385]      # [1,256] ones
        eps_ap = cst[0:1, 385:386]        # [1,1] EPS

        et0 = epool.tile([128, 6, CH], F16, tag="et", name="et")
        nc.sync.dma_start(et0[:], edge_p[0])

        for l in range(layers):
            wm = wpool.tile([128, WMC], F16, tag="wm")
            nc.sync.dma_start(wm[:, 0:O_W1], wm_p[l][:, 0:O_W1])
            nc.gpsimd.dma_start(wm[:, O_W1:O_DW0], wm_p[l][:, O_W1:O_DW0])
            nc.gpsimd.dma_start(wm[:, O_DW0:WMC], wm_p[l][:, O_DW0:WMC])
            wf = wpool.tile([128, WFC], F32, tag="wf")
            nc.gpsimd.dma_start(wf[:], wf_p[l])
            lnw = wpool.tile([1, 1536], F32R, tag="lnw")
            nc.gpsimd.dma_start(lnw[:], ln_p[l])

            # --- xw_b = x @ W0x + b0  (fp16, feature-major), replicated x4 ---
            xb = xpool.tile([128, 3, NLOC], F16, tag="xb")
            nc.vector.tensor_copy(xb[:], x[:].bitcast(F32))
            xwp = mm.tile([128, 3, 512], F32, tag="mm")
            for mt in range(3):
                for kt in range(3):
                    nc.tensor.matmul(
                        xwp[:, mt, 0:256],
                        wm[:, O_W0X + kt * 384 + mt * 128: O_W0X + kt * 384 + (mt + 1) * 128],
                        xb[:, kt, :],
                        start=(kt == 0), stop=(kt == 2))
            xw4 = work1.tile([128, 3, CH], F16, tag="xw4")
            for mt in range(3):
                nc.scalar.activation(xw4[:, mt, 0:256], xwp[:, mt, 0:256],
                                     AF.Identity,
                                     bias=wf[:, O_B0 + mt:O_B0 + mt + 1])
            for r in range(1, 4):
                nc.vector.tensor_copy(xw4[:, :, r * 256:(r + 1) * 256],
                                      xw4[:, :, 0:256])

            # b2s = b2 * (K/SCALE)
            b2s = small.tile([128, 3], F32, tag="b2s")
            nc.vector.tensor_scalar_mul(b2s[:], wf[:, O_B2:O_B2 + 3], K / SCALE)

            # --- k loop: 24 software-pipelined half-iterations of 512
            # tokens. Emission order per step i keeps the PE stream dense:
            # hp(i) MMs run while h0g(i-1) act / h1g(i-2) acts complete, so
            # the PSUM WAR chain never stalls the PE. agg for chunk c is
            # delayed two steps so its hsum input is long done.
            agg = aggp.tile([128, 768], F32, tag="agg")
            NH = 2 * NCH
            et_t = [None] * NCH
            h0g_t = [None] * NH
            h1g_t = [None] * NH
            sh_t = [None] * NH

            def emit_h0(i):
                cc, h = i // 2, i % 2
                if h == 0:
                    if l == 0 and cc == 0:
                        et_t[cc] = et0
                    else:
                        et_t[cc] = epool.tile([128, 6, CH], F16, tag="et",
                                              name="et")
                        nc.sync.dma_start(et_t[cc][:], edge_p[cc])
                et = et_t[cc]
                hp = mm.tile([128, 3, 512], F32, tag="mm", name="hp")
                for mt in range(3):
                    for kt in range(6):
                        off = O_W0EN + kt * 384 + mt * 128
                        nc.tensor.matmul(
                            hp[:, mt, :], wm[:, off:off + 128],
                            et[:, kt, h * 512:(h + 1) * 512],
                            start=(kt == 0), stop=(kt == 5))
                h0pre = hpool.tile([128, 3, 512], F16, tag=f"h0pre{i % 2}",
                                   name="h0pre")
                nc.vector.tensor_add(h0pre[:], hp[:],
                                     xw4[:, :, h * 512:(h + 1) * 512])
                h0g_t[i] = hpool.tile([128, 3, 512], F16, tag=f"h0g{i % 2}",
                                      name="h0g")
                nc.scalar.activation(h0g_t[i][:].rearrange("p a b -> p (a b)"),
                                     h0pre[:].rearrange("p a b -> p (a b)"),
                                     act)

            def emit_h1(i):
                h1p = mm.tile([128, 3, 512], F32, tag="mm", name="h1p")
                for mt in range(3):
                    for kt in range(3):
                        off = O_W1 + kt * 384 + mt * 128
                        nc.tensor.matmul(
                            h1p[:, mt, :], wm[:, off:off + 128],
                            h0g_t[i][:, kt, :],
                            start=(kt == 0), stop=(kt == 2))
                h1g = hpool.tile([128, 3, 512], F16, tag=f"h1g{i % 2}",
                                 name="h1g")
                for mt in range(3):
                    nc.scalar.activation(h1g[:, mt, :], h1p[:, mt, :], act,
                                         bias=wf[:, O_B1 + mt:O_B1 + mt + 1])
                sh_t[i] = xg1.tile([128, 3, 256], F16, tag=f"sh{i % 5}",
                                     name="sh")
                nc.vector.tensor_add(sh_t[i][:], h1g[:, :, 0:256],
                                     h1g[:, :, 256:512])

            def emit_agg(p):
                # p indexes a pair of chunks (8 k-groups pre-summed on DVE).
                # mt=1 shares a PSUM bank with mt=0: start=True pends the
                # whole 2KB zero region, so mt=1 must NOT issue start.
                ha = xg1.tile([128, 3, 256], F16, tag="ha", name="ha")
                nc.vector.tensor_add(ha[:], sh_t[4 * p][:], sh_t[4 * p + 1][:])
                hb = xg1.tile([128, 3, 256], F16, tag="hb", name="hb")
                nc.vector.tensor_add(hb[:], sh_t[4 * p + 2][:],
                                     sh_t[4 * p + 3][:])
                hs = xg1.tile([128, 3, 256], F16, tag="hs", name="hs")
                nc.vector.tensor_add(hs[:], ha[:], hb[:])
                for mt in range(3):
                    for kt in range(3):
                        off = O_W2 + kt * 384 + mt * 128
                        nc.tensor.matmul(
                            agg[:, mt * 256:(mt + 1) * 256],
                            wm[:, off:off + 128], hs[:, kt, :],
                            start=(p == 0 and kt == 0 and mt != 1),
                            stop=(p == NCH // 2 - 1 and kt == 2),
                            skip_group_check=True)

            for i in range(NH + 2):
                if i < NH:
                    emit_h0(i)
                if 1 <= i <= NH:
                    emit_h1(i - 1)
                if i >= 5 and (i - 5) % 4 == 0 and (i - 5) // 4 < NCH // 2:
                    emit_agg((i - 5) // 4)

            # --- x1_pre = x + agg + b2*K/SCALE (W2 is host-scaled by 1/30)
            dum = small.tile([1, 1], F32, tag="dum")
            nc.scalar.activation(dum[:], cst[0:1, 0:1].bitcast(F32), AF.Sqrt)
            t1 = work1.tile([128, 3, 256], F32, tag="t1")
            for mt in range(3):
                nc.scalar.activation(t1[:, mt, :], agg[:, mt * 256:(mt + 1) * 256],
                                     AF.Identity, bias=b2s[:, mt:mt + 1])
            x1p = work1.tile([128, 3, 256], F32R, tag="x1p")
            nc.vector.tensor_add(x1p[:], x[:].bitcast(F32), t1[:])

            def layernorm(src, ln_i, masked):
                """src: [128,3,256] F32R tile -> returns new [128,3,256] tile."""
                sq = work1.tile([128, 3, 256], F32R, tag="sq")
                nc.vector.tensor_mul(sq[:], src[:].bitcast(F32),
                                     src[:].bitcast(F32))
                st = mm.tile([128, 3, 512], F32, tag="mm")
                for kt in range(3):
                    nc.tensor.matmul(st[0:1, 0, 0:256], ones_col, src[:, kt, :],
                                     start=(kt == 0), stop=(kt == 2))
                for kt in range(3):
                    nc.tensor.matmul(st[0:1, 0, 256:512], ones_col, sq[:, kt, :],
                                     start=(kt == 0), stop=(kt == 2))
                # st[0,0:256]=mean, st[0,256:512]=E[x^2] (ones_col is 1/NF)
                m_ap = small.tile([1, 256], F32, tag="m_ap")
                nc.vector.tensor_copy(m_ap[:], st[0:1, 0, 0:256])
                m_ap = m_ap[:]
                msq = small.tile([1, 256], F32, tag="msq")
                nc.vector.tensor_mul(msq[:], m_ap, m_ap)
                var = small.tile([1, 256], F32, tag="var")
                nc.vector.tensor_sub(var[:], st[0:1, 0, 256:512], msq[:])
                sd = small.tile([1, 256], F32, tag="sd")
                nc.scalar.activation(sd[:], var[:], AF.Sqrt,
                                     bias=eps_ap.bitcast(F32))
                # rv rows (f32r): [0:256]=rstd(*mask), [256:512]=-m*rstd(*mask),
                # [512:768]= ones or mask
                rv = small.tile([1, 768], F32R, tag="rv")
                if masked:
                    rstd = small.tile([1, 256], F32, tag="rstd")
                    nc.vector.reciprocal_approx_fast(rstd[:], sd[:])
                    nc.vector.tensor_mul(rv[0:1, 0:256], rstd[:], maskt[:])
                    nmr = small.tile([1, 256], F32, tag="nmr")
                    nc.vector.scalar_tensor_tensor(
                        nmr[:], m_ap, -1.0, rstd[:],
                        op0=mybir.AluOpType.mult, op1=mybir.AluOpType.mult)
                    nc.vector.tensor_mul(rv[0:1, 256:512], nmr[:], maskt[:])

                    nc.vector.tensor_copy(rv[0:1, 512:768], maskt[:])
                else:
                    rstd = small.tile([1, 256], F32, tag="rstd")
                    nc.vector.reciprocal_approx_fast(rstd[:], sd[:])
                    nc.vector.tensor_copy(rv[0:1, 0:256], rstd[:])
                    nc.vector.scalar_tensor_tensor(
                        rv[0:1, 256:512], m_ap, -1.0, rstd[:],
                        op0=mybir.AluOpType.mult, op1=mybir.AluOpType.mult)
                    nc.vector.tensor_copy(rv[0:1, 512:768], ones_row.bitcast(F32))
                stp = mm.tile([128, 3, 512], F32, tag="mm")
                for mt in range(3):
                    woff = ln_i * 384 + mt * 128
                    nc.tensor.matmul(stp[:, mt, 0:256], lnw[0:1, woff:woff + 128],
                                     rv[0:1, 0:256], start=True, stop=True)
                    nc.tensor.matmul(stp[:, mt, 256:512],
                                     lnw[0:1, 768 + woff:768 + woff + 128],
                                     rv[0:1, 512:768], start=True, stop=False)
                    nc.tensor.matmul(stp[:, mt, 256:512], lnw[0:1, woff:woff + 128],
                                     rv[0:1, 256:512], start=False, stop=True)
                outt = xpool.tile([128, 3, 256], F32R, tag="lnout")
                for mt in range(3):
                    nc.vector.tensor_mul(outt[:, mt, :], src[:, mt, :].bitcast(F32),
                                         stp[:, mt, 0:256])
                    nc.vector.tensor_add(outt[:, mt, :], outt[:, mt, :].bitcast(F32),
                                         stp[:, mt, 256:512])
                return outt

            x1 = layernorm(x1p, 0, masked=False)
            x1b = xpool.tile([128, 3, 256], F16, tag="x1b")
            nc.vector.tensor_copy(x1b[:], x1[:].bitcast(F32))

            # --- dense MLP: d0 = gelu(x1 @ dw0 + db0); d1 = d0 @ dw1 + db1 ---
            d0g = work1.tile([128, 12, 256], F16, tag="d0g")
            for half in range(2):
                dp = mm.tile([128, 3, 512], F32, tag="mm")
                for m6 in range(6):
                    mt = half * 6 + m6
                    reg = dp[:, m6 // 2, (m6 % 2) * 256:(m6 % 2) * 256 + 256]
                    for kt in range(3):
                        nc.tensor.matmul(
                            reg,
                            wm[:, O_DW0 + kt * 1536 + mt * 128: O_DW0 + kt * 1536 + (mt + 1) * 128],
                            x1b[:, kt, :],
                            start=(kt == 0), stop=(kt == 2))
                    nc.scalar.activation(d0g[:, mt, :], reg, act,
                                         bias=wf[:, O_DB0 + mt:O_DB0 + mt + 1])
            d1p = mm.tile([128, 3, 512], F32, tag="mm")
            for kh in range(2):
                for mt in range(3):
                    for k6 in range(6):
                        kt = kh * 6 + k6
                        nc.tensor.matmul(
                            d1p[:, mt, 0:256],
                            wm[:, O_DW1 + kt * 384 + mt * 128: O_DW1 + kt * 384 + (mt + 1) * 128],
                            d0g[:, kt, :],
                            start=(kt == 0), stop=(kt == 11),
                            skip_group_check=True)
            dum2 = small.tile([1, 1], F32, tag="dum2")
            nc.scalar.activation(dum2[:], cst[0:1, 0:1].bitcast(F32), AF.Sqrt)
            t2 = work1.tile([128, 3, 256], F32, tag="t2")
            for mt in range(3):
                nc.scalar.activation(t2[:, mt, :], d1p[:, mt, 0:256], AF.Identity,
                                     bias=wf[:, O_DB1 + mt:O_DB1 + mt + 1])
            x2p = work1.tile([128, 3, 256], F32R, tag="x2p")
            nc.vector.tensor_add(x2p[:], x1[:].bitcast(F32), t2[:])

            x = layernorm(x2p, 1, masked=True)

        nc.sync.dma_start(out_p[:], x[:].bitcast(F32))

    nc.finalize()
    return nc


def _get_nc():
    if "nc" not in _NC_CACHE:
        _NC_CACHE["nc"] = _emit()
    return _NC_CACHE["nc"]


def _fm(w):
    """[in, out] fp32 -> [128, n_kt*out] (feature-major lhsT blob columns)."""
    i, o = w.shape
    return np.ascontiguousarray(
        w.reshape(i // 128, 128, o).transpose(1, 0, 2).reshape(128, -1))


def _marshal(inputs):
    nf = np.asarray(inputs["node_features"], np.float32)
    ef = np.asarray(inputs["edge_features"], np.float32)
    idx = np.asarray(inputs["neighbor_indices"])
    mask = np.asarray(inputs["mask"], np.float32)

    # replicated tensors
    wm = np.empty((L, 128, WMC), np.float16)
    wf = np.empty((L, 128, WFC), np.float32)
    lnpk = np.empty((L, 1, 1536), np.float32)
    for l in range(L):
        w0 = np.asarray(inputs["msg_w0"], np.float32)[l]
        cols = [
            _fm(w0[0:384]), _fm(w0[384:768]), _fm(w0[1152:1536]),
            _fm(np.asarray(inputs["msg_w1"], np.float32)[l]),
            _fm(np.asarray(inputs["msg_w2"], np.float32)[l] / SCALE),
            _fm(np.asarray(inputs["dense_w0"], np.float32)[l]),
            _fm(np.asarray(inputs["dense_w1"], np.float32)[l]),
        ]
        wm[l] = np.concatenate(cols, axis=1).astype(np.float16)
        fcols = [
            np.asarray(inputs["msg_b0"], np.float32)[l].reshape(3, 128).T,
            np.asarray(inputs["msg_b1"], np.float32)[l].reshape(3, 128).T,
            np.asarray(inputs["msg_b2"], np.float32)[l].reshape(3, 128).T,
            np.asarray(inputs["dense_b0"], np.float32)[l].reshape(12, 128).T,
            np.asarray(inputs["dense_b1"], np.float32)[l].reshape(3, 128).T,
        ]
        wf[l] = np.concatenate(fcols, axis=1)
        lnpk[l, 0] = np.concatenate([
            np.asarray(inputs["ln1_w"], np.float32)[l],
            np.asarray(inputs["ln2_w"], np.float32)[l],
            np.asarray(inputs["ln1_b"], np.float32)[l],
            np.asarray(inputs["ln2_b"], np.float32)[l]])
    consts = np.zeros((128, 386), np.float32)
    consts[:, 0] = 1.0 / NF
    consts[:, 1:129] = np.eye(128, dtype=np.float32)
    consts[0, 129:385] = 1.0
    consts[0, 385] = EPS

    nf16 = nf.astype(np.float16)
    in_maps = []
    for c in range(NCORES):
        lo = slice(c * NLOC, (c + 1) * NLOC)
        el = ef[lo]                                        # [256,48,384]
        E = el.transpose(1, 0, 2).reshape(T, 384).astype(np.float16)  # k-major
        idx_k = np.ascontiguousarray(idx[lo].T).reshape(T)     # k-major values
        nfg = nf16[idx_k]                                  # [T,384] host gather
        comb = np.concatenate([E, nfg], axis=1)            # [T,768]
        edge = np.ascontiguousarray(
            comb.reshape(NCH, CH, 6, 128).transpose(0, 3, 2, 1))
        x0 = np.ascontiguousarray(
            nf[lo].reshape(NLOC, 3, 128).transpose(2, 1, 0))   # [128,3,256]
        in_maps.append(dict(
            edge=edge, wm=wm, wf=wf, lnpk=lnpk,
            consts=consts, x0=x0,
            mask=np.ascontiguousarray(mask[lo])[None, :]))
    return in_maps


def _unshard(results):
    out = np.empty((N, NF), np.float32)
    for c in range(NCORES):
        xfm = results[c]["out_x"]                          # [128,3,256]
        out[c * NLOC:(c + 1) * NLOC] = xfm.transpose(2, 1, 0).reshape(NLOC, NF)
    return out


def kernel(**inputs):
    nc = _get_nc()
    in_maps = _marshal(inputs)
    res = run_bass_kernel_spmd(nc, in_maps, list(range(NCORES)), trace=False)
    return _unshard(res.results)


# revision 17
# speedup vs baseline: 1.1926x; 1.0058x over previous
"""Trainium2 Bass kernel for nn_Decoder_60627758350737 (GNN message passing).

Sharding: node dim N=2048 split across 8 cores (256 nodes each). All MLP
weights replicated; no collectives.

The reference computes layer_edge_features once from the *initial* node
features, and neighbor_indices is a host-visible input — so the neighbor
gather nf0[idx] is done on the host during marshaling and shipped
feature-major alongside the edge features ([128, 6, 1024] fp16 per chunk:
kt 0-2 edges, kt 3-5 gathered nf0). The device applies [W0e | W0n] as a
768-deep contraction inside h0 — no on-device gather at all.

All message-MLP matmuls run in fp16 (full PE rate + fast weight load;
fp32-HIGH mode is ~1.3x slower per column and blocks FWL). Layout is
feature-major ([feature_part, token]); tokens are processed in 12 chunks
of 1024 (k-major: t = k*256 + n). Per chunk (paired 512-col matmuls so
each weight block loads once):
  h0   = gelu(W0en.T@[edge|nfg] + I.T@xw_b)   (xw_b = x@W0x + b0)
  h1   = gelu(W1.T@h0 + b1)
  hsum = sum of the 4 k-group slices of h1 (vector, fp16)
  agg += W2.T@hsum                            (PSUM accumulate across chunks)
then x1 = LN(x + agg/30 + 1.6*b2); x = mask*LN(x1 + dense(x1)) in fp32.
LN mean/var via ones-vector matmuls and rank-1 outer-product apply.
NOTE: agg's three 256-col regions share PSUM banks; only the first region
per bank may issue start=True (bank-wide pending-zero semantics).
"""
import numpy as np
import concourse.bass as bass
import concourse.bacc as bacc
import concourse.mybir as mybir
from concourse import tile
from concourse.bass_utils import run_bass_kernel_spmd
from contextlib import ExitStack

F32 = mybir.dt.float32
F32R = mybir.dt.float32r
F16 = mybir.dt.float16
AF = mybir.ActivationFunctionType

N, K, NF, L = 2048, 48, 384, 3
NCORES = 8
NLOC = N // NCORES            # 256
T = NLOC * K                  # 12288 tokens (k-major: t = k*256 + n)
CH = 1024                     # chunk (tokens)
NCH = T // CH                 # 12
SCALE = 30.0
EPS = 1e-5

# fp16 weight blob column offsets (per layer, [128, WMC])
O_W0X = 0
O_W0EN = 1152
O_W1 = 3456
O_W2 = 4608
O_DW0 = 5760
O_DW1 = 10368
WMC = 14976
# f32 bias blob columns (per layer, [128, WFC])
O_B0 = 0
O_B1 = 3
O_B2 = 6
O_DB0 = 9
O_DB1 = 21
WFC = 24

_NC_CACHE = {}


def _emit(act=None, layers=L):
    act = AF.Gelu if act is None else act
    nc = bacc.Bacc()
    edge_p = nc.declare_dram_parameter("edge", [NCH, 128, 6, CH], F16, isOutput=False)
    wm_p = nc.declare_dram_parameter("wm", [L, 128, WMC], F16, isOutput=False)
    wf_p = nc.declare_dram_parameter("wf", [L, 128, WFC], F32, isOutput=False)
    ln_p = nc.declare_dram_parameter("lnpk", [L, 1, 1536], F32R, isOutput=False)
    cst_p = nc.declare_dram_parameter("consts", [128, 386], F32R, isOutput=False)
    x0_p = nc.declare_dram_parameter("x0", [128, 3, NLOC], F32R, isOutput=False)
    mask_p = nc.declare_dram_parameter("mask", [1, NLOC], F32, isOutput=False)
    out_p = nc.declare_dram_parameter("out_x", [128, 3, NLOC], F32, isOutput=True)

    with tile.TileContext(nc) as tc, ExitStack() as ctx:
        wpool = ctx.enter_context(tc.tile_pool(name="w", bufs=2))
        epool = ctx.enter_context(tc.tile_pool(name="ep", bufs=2))
        xgpool = ctx.enter_context(tc.tile_pool(name="xgp", bufs=2))
        xg1 = ctx.enter_context(tc.tile_pool(name="xg1", bufs=1))
        hpool = ctx.enter_context(tc.tile_pool(name="hp", bufs=2))
        work1 = ctx.enter_context(tc.tile_pool(name="work1", bufs=1))
        xpool = ctx.enter_context(tc.tile_pool(name="xp", bufs=2))
        small = ctx.enter_context(tc.tile_pool(name="small", bufs=1))
        mm = ctx.enter_context(tc.tile_pool(name="mm", bufs=2, space="PSUM"))
        aggp = ctx.enter_context(tc.tile_pool(name="aggp", bufs=1, space="PSUM"))

        # --- one-time loads ---
        cst = small.tile([128, 386], F32R, tag="cst")
        nc.gpsimd.dma_start(cst[:], cst_p[:])
        maskt = small.tile([1, NLOC], F32, tag="maskt")
        nc.gpsimd.dma_start(maskt[:], mask_p[:])
        x = xpool.tile([128, 3, NLOC], F32R, tag="x")
        nc.gpsimd.dma_start(x[:], x0_p[:])

        ones_col = cst[:, 0:1]            # [128,1] 1/NF (stats lhsT -> means)
        ones_row = cst[0:1, 129:385]      # [1,256] ones
        eps_ap = cst[0:1, 385:386]        # [1,1] EPS

        et0 = epool.tile([128, 6, CH], F16, tag="et", name="et")
        nc.sync.dma_start(et0[:], edge_p[0])
        xb_t = [None]

        for l in range(layers):
            wm = wpool.tile([128, WMC], F16, tag="wm")
            nc.sync.dma_start(wm[:, 0:O_W1], wm_p[l][:, 0:O_W1])
            nc.gpsimd.dma_start(wm[:, O_W1:O_DW0], wm_p[l][:, O_W1:O_DW0])
            nc.gpsimd.dma_start(wm[:, O_DW0:WMC], wm_p[l][:, O_DW0:WMC])
            wf = wpool.tile([128, WFC], F32, tag="wf")
            nc.gpsimd.dma_start(wf[:], wf_p[l])
            lnw = wpool.tile([1, 1536], F32R, tag="lnw")
            nc.gpsimd.dma_start(lnw[:], ln_p[l])

            # --- xw_b = x @ W0x + b0  (fp16, feature-major), replicated x4 ---
            if xb_t[0] is None:
                xb = xpool.tile([128, 3, NLOC], F16, tag="xb")
                nc.vector.tensor_copy(xb[:], x[:].bitcast(F32))
            else:
                xb = xb_t[0]
            xwp = mm.tile([128, 3, 512], F32, tag="mm")
            for mt in range(3):
                for kt in range(3):
                    nc.tensor.matmul(
                        xwp[:, mt, 0:256],
                        wm[:, O_W0X + kt * 384 + mt * 128: O_W0X + kt * 384 + (mt + 1) * 128],
                        xb[:, kt, :],
                        start=(kt == 0), stop=(kt == 2))
            xw4 = work1.tile([128, 3, CH], F16, tag="xw4")
            for mt in range(3):
                nc.scalar.activation(xw4[:, mt, 0:256], xwp[:, mt, 0:256],
                                     AF.Identity,
                                     bias=wf[:, O_B0 + mt:O_B0 + mt + 1])
            for r in range(1, 4):
                nc.vector.tensor_copy(xw4[:, :, r * 256:(r + 1) * 256],
                                      xw4[:, :, 0:256])

            # b2s = b2 * (K/SCALE)
            b2s = small.tile([128, 3], F32, tag="b2s")
            nc.vector.tensor_scalar_mul(b2s[:], wf[:, O_B2:O_B2 + 3], K / SCALE)

            # --- k loop: 24 software-pipelined half-iterations of 512
            # tokens. Emission order per step i keeps the PE stream dense:
            # hp(i) MMs run while h0g(i-1) act / h1g(i-2) acts complete, so
            # the PSUM WAR chain never stalls the PE. agg for chunk c is
            # delayed two steps so its hsum input is long done.
            agg = aggp.tile([128, 768], F32, tag="agg")
            NH = 2 * NCH
            et_t = [None] * NCH
            h0g_t = [None] * NH
            h1g_t = [None] * NH
            sh_t = [None] * NH

            def emit_h0(i):
                cc, h = i // 2, i % 2
                if h == 0:
                    if l == 0 and cc == 0:
                        et_t[cc] = et0
                    else:
                        et_t[cc] = epool.tile([128, 6, CH], F16, tag="et",
                                              name="et")
                        nc.sync.dma_start(et_t[cc][:], edge_p[cc])
                et = et_t[cc]
                hp = mm.tile([128, 3, 512], F32, tag="mm", name="hp")
                for mt in range(3):
                    for kt in range(6):
                        off = O_W0EN + kt * 384 + mt * 128
                        nc.tensor.matmul(
                            hp[:, mt, :], wm[:, off:off + 128],
                            et[:, kt, h * 512:(h + 1) * 512],
                            start=(kt == 0), stop=(kt == 5))
                h0pre = hpool.tile([128, 3, 512], F16, tag=f"h0pre{i % 2}",
                                   name="h0pre")
                nc.vector.tensor_add(h0pre[:], hp[:],
                                     xw4[:, :, h * 512:(h + 1) * 512])
                h0g_t[i] = hpool.tile([128, 3, 512], F16, tag=f"h0g{i % 2}",
                                      name="h0g")
                nc.scalar.activation(h0g_t[i][:].rearrange("p a b -> p (a b)"),
                                     h0pre[:].rearrange("p a b -> p (a b)"),
                                     act)

            def emit_h1(i):
                h1p = mm.tile([128, 3, 512], F32, tag="mm", name="h1p")
                for mt in range(3):
                    for kt in range(3):
                        off = O_W1 + kt * 384 + mt * 128
                        nc.tensor.matmul(
                            h1p[:, mt, :], wm[:, off:off + 128],
                            h0g_t[i][:, kt, :],
                            start=(kt == 0), stop=(kt == 2))
                h1g = hpool.tile([128, 3, 512], F16, tag=f"h1g{i % 2}",
                                 name="h1g")
                for mt in range(3):
                    nc.scalar.activation(h1g[:, mt, :], h1p[:, mt, :], act,
                                         bias=wf[:, O_B1 + mt:O_B1 + mt + 1])
                sh_t[i] = xg1.tile([128, 3, 256], F16, tag=f"sh{i % 5}",
                                     name="sh")
                nc.vector.tensor_add(sh_t[i][:], h1g[:, :, 0:256],
                                     h1g[:, :, 256:512])

            def emit_agg(p):
                # p indexes a pair of chunks (8 k-groups pre-summed on DVE).
                # mt=1 shares a PSUM bank with mt=0: start=True pends the
                # whole 2KB zero region, so mt=1 must NOT issue start.
                ha = xg1.tile([128, 3, 256], F16, tag="ha", name="ha")
                nc.vector.tensor_add(ha[:], sh_t[4 * p][:], sh_t[4 * p + 1][:])
                hb = xg1.tile([128, 3, 256], F16, tag="hb", name="hb")
                nc.vector.tensor_add(hb[:], sh_t[4 * p + 2][:],
                                     sh_t[4 * p + 3][:])
                hs = xg1.tile([128, 3, 256], F16, tag="hs", name="hs")
                nc.vector.tensor_add(hs[:], ha[:], hb[:])
                for mt in range(3):
                    for kt in range(3):
                        off = O_W2 + kt * 384 + mt * 128
                        nc.tensor.matmul(
                            agg[:, mt * 256:(mt + 1) * 256],
                            wm[:, off:off + 128], hs[:, kt, :],
                            start=(p == 0 and kt == 0 and mt != 1),
                            stop=(p == NCH // 2 - 1 and kt == 2),
                            skip_group_check=True)

            for i in range(NH + 2):
                if i < NH:
                    emit_h0(i)
                if 1 <= i <= NH:
                    emit_h1(i - 1)
                if i >= 5 and (i - 5) % 4 == 0 and (i - 5) // 4 < NCH // 2:
                    emit_agg((i - 5) // 4)

            # --- x1_pre = x + agg + b2*K/SCALE (W2 is host-scaled by 1/30)
            dum = small.tile([1, 1], F32, tag="dum")
            nc.scalar.activation(dum[:], cst[0:1, 0:1].bitcast(F32), AF.Sqrt)
            t1 = work1.tile([128, 3, 256], F32, tag="t1")
            for mt in range(3):
                nc.scalar.activation(t1[:, mt, :], agg[:, mt * 256:(mt + 1) * 256],
                                     AF.Identity, bias=b2s[:, mt:mt + 1])
            x1p = work1.tile([128, 3, 256], F32R, tag="x1p")
            nc.vector.tensor_add(x1p[:], x[:].bitcast(F32), t1[:])

            def layernorm(src, ln_i, masked):
                """src: [128,3,256] F32R tile -> returns new [128,3,256] tile."""
                sq = work1.tile([128, 3, 256], F32R, tag="sq")
                nc.vector.tensor_mul(sq[:], src[:].bitcast(F32),
                                     src[:].bitcast(F32))
                st = mm.tile([128, 3, 512], F32, tag="mm")
                for kt in range(3):
                    nc.tensor.matmul(st[0:1, 0, 0:256], ones_col, src[:, kt, :],
                                     start=(kt == 0), stop=(kt == 2))
                for kt in range(3):
                    nc.tensor.matmul(st[0:1, 0, 256:512], ones_col, sq[:, kt, :],
                                     start=(kt == 0), stop=(kt == 2))
                # st[0,0:256]=mean, st[0,256:512]=E[x^2] (ones_col is 1/NF)
                m_ap = small.tile([1, 256], F32, tag="m_ap")
                nc.vector.tensor_copy(m_ap[:], st[0:1, 0, 0:256])
                m_ap = m_ap[:]
                msq = small.tile([1, 256], F32, tag="msq")
                nc.vector.tensor_mul(msq[:], m_ap, m_ap)
                var = small.tile([1, 256], F32, tag="var")
                nc.vector.tensor_sub(var[:], st[0:1, 0, 256:512], msq[:])
                sd = small.tile([1, 256], F32, tag="sd")
                nc.scalar.activation(sd[:], var[:], AF.Sqrt,
                                     bias=eps_ap.bitcast(F32))
                # rv rows (f32r): [0:256]=rstd(*mask), [256:512]=-m*rstd(*mask),
                # [512:768]= ones or mask
                rv = small.tile([1, 768], F32R, tag="rv")
                if masked:
                    rstd = small.tile([1, 256], F32, tag="rstd")
                    nc.vector.reciprocal_approx_fast(rstd[:], sd[:])
                    nc.vector.tensor_mul(rv[0:1, 0:256], rstd[:], maskt[:])
                    nmr = small.tile([1, 256], F32, tag="nmr")
                    nc.vector.scalar_tensor_tensor(
                        nmr[:], m_ap, -1.0, rstd[:],
                        op0=mybir.AluOpType.mult, op1=mybir.AluOpType.mult)
                    nc.vector.tensor_mul(rv[0:1, 256:512], nmr[:], maskt[:])

                    nc.vector.tensor_copy(rv[0:1, 512:768], maskt[:])
                else:
                    rstd = small.tile([1, 256], F32, tag="rstd")
                    nc.vector.reciprocal_approx_fast(rstd[:], sd[:])
                    nc.vector.tensor_copy(rv[0:1, 0:256], rstd[:])
                    nc.vector.scalar_tensor_tensor(
                        rv[0:1, 256:512], m_ap, -1.0, rstd[:],
                        op0=mybir.AluOpType.mult, op1=mybir.AluOpType.mult)
                    nc.vector.tensor_copy(rv[0:1, 512:768], ones_row.bitcast(F32))
                stp = mm.tile([128, 3, 512], F32, tag="mm")
                for mt in range(3):
                    woff = ln_i * 384 + mt * 128
                    nc.tensor.matmul(stp[:, mt, 0:256], lnw[0:1, woff:woff + 128],
                                     rv[0:1, 0:256], start=True, stop=True)
                    nc.tensor.matmul(stp[:, mt, 256:512],
                                     lnw[0:1, 768 + woff:768 + woff + 128],
                                     rv[0:1, 512:768], start=True, stop=False)
                    nc.tensor.matmul(stp[:, mt, 256:512], lnw[0:1, woff:woff + 128],
                                     rv[0:1, 256:512], start=False, stop=True)
                outt = xpool.tile([128, 3, 256], F32R, tag="lnout")
                out16 = xpool.tile([128, 3, 256], F16, tag="lnout16")
                for mt in range(3):
                    nc.vector.tensor_mul(outt[:, mt, :], src[:, mt, :].bitcast(F32),
                                         stp[:, mt, 0:256])
                    nc.vector.tensor_add(outt[:, mt, :], outt[:, mt, :].bitcast(F32),
                                         stp[:, mt, 256:512])
                    nc.vector.tensor_copy(out16[:, mt, :],
                                          outt[:, mt, :].bitcast(F32))
                return outt, out16

            x1, x1b = layernorm(x1p, 0, masked=False)

            # --- dense MLP: d0 = gelu(x1 @ dw0 + db0); d1 = d0 @ dw1 + db1 ---
            d0g = work1.tile([128, 12, 256], F16, tag="d0g")
            for half in range(2):
                dp = mm.tile([128, 3, 512], F32, tag="mm")
                for m6 in range(6):
                    mt = half * 6 + m6
                    reg = dp[:, m6 // 2, (m6 % 2) * 256:(m6 % 2) * 256 + 256]
                    for kt in range(3):
                        nc.tensor.matmul(
                            reg,
                            wm[:, O_DW0 + kt * 1536 + mt * 128: O_DW0 + kt * 1536 + (mt + 1) * 128],
                            x1b[:, kt, :],
                            start=(kt == 0), stop=(kt == 2))
                    nc.scalar.activation(d0g[:, mt, :], reg, act,
                                         bias=wf[:, O_DB0 + mt:O_DB0 + mt + 1])
            d1p = mm.tile([128, 3, 512], F32, tag="mm")
            for kh in range(2):
                for mt in range(3):
                    for k6 in range(6):
                        kt = kh * 6 + k6
                        nc.tensor.matmul(
                            d1p[:, mt, 0:256],
                            wm[:, O_DW1 + kt * 384 + mt * 128: O_DW1 + kt * 384 + (mt + 1) * 128],
                            d0g[:, kt, :],
                            start=(kt == 0), stop=(kt == 11),
                            skip_group_check=True)
            dum2 = small.tile([1, 1], F32, tag="dum2")
            nc.scalar.activation(dum2[:], cst[0:1, 0:1].bitcast(F32), AF.Sqrt)
            t2 = work1.tile([128, 3, 256], F32, tag="t2")
            for mt in range(3):
                nc.scalar.activation(t2[:, mt, :], d1p[:, mt, 0:256], AF.Identity,
                                     bias=wf[:, O_DB1 + mt:O_DB1 + mt + 1])
            x2p = work1.tile([128, 3, 256], F32R, tag="x2p")
            nc.vector.tensor_add(x2p[:], x1[:].bitcast(F32), t2[:])

            x, xb_next = layernorm(x2p, 1, masked=True)
            xb_t[0] = xb_next

        nc.sync.dma_start(out_p[:], x[:].bitcast(F32))

    nc.finalize()
    return nc


def _get_nc():
    if "nc" not in _NC_CACHE:
        _NC_CACHE["nc"] = _emit()
    return _NC_CACHE["nc"]


def _fm(w):
    """[in, out] fp32 -> [128, n_kt*out] (feature-major lhsT blob columns)."""
    i, o = w.shape
    return np.ascontiguousarray(
        w.reshape(i // 128, 128, o).transpose(1, 0, 2).reshape(128, -1))


def _marshal(inputs):
    nf = np.asarray(inputs["node_features"], np.float32)
    ef = np.asarray(inputs["edge_features"], np.float32)
    idx = np.asarray(inputs["neighbor_indices"])
    mask = np.asarray(inputs["mask"], np.float32)

    # replicated tensors
    wm = np.empty((L, 128, WMC), np.float16)
    wf = np.empty((L, 128, WFC), np.float32)
    lnpk = np.empty((L, 1, 1536), np.float32)
    for l in range(L):
        w0 = np.asarray(inputs["msg_w0"], np.float32)[l]
        cols = [
            _fm(w0[0:384]), _fm(w0[384:768]), _fm(w0[1152:1536]),
            _fm(np.asarray(inputs["msg_w1"], np.float32)[l]),
            _fm(np.asarray(inputs["msg_w2"], np.float32)[l] / SCALE),
            _fm(np.asarray(inputs["dense_w0"], np.float32)[l]),
            _fm(np.asarray(inputs["dense_w1"], np.float32)[l]),
        ]
        wm[l] = np.concatenate(cols, axis=1).astype(np.float16)
        fcols = [
            np.asarray(inputs["msg_b0"], np.float32)[l].reshape(3, 128).T,
            np.asarray(inputs["msg_b1"], np.float32)[l].reshape(3, 128).T,
            np.asarray(inputs["msg_b2"], np.float32)[l].reshape(3, 128).T,
            np.asarray(inputs["dense_b0"], np.float32)[l].reshape(12, 128).T,
            np.asarray(inputs["dense_b1"], np.float32)[l].reshape(3, 128).T,
        ]
        wf[l] = np.concatenate(fcols, axis=1)
        lnpk[l, 0] = np.concatenate([
            np.asarray(inputs["ln1_w"], np.float32)[l],
            np.asarray(inputs["ln2_w"], np.float32)[l],
            np.asarray(inputs["ln1_b"], np.float32)[l],
            np.asarray(inputs["ln2_b"], np.float32)[l]])
    consts = np.zeros((128, 386), np.float32)
    consts[:, 0] = 1.0 / NF
    consts[:, 1:129] = np.eye(128, dtype=np.float32)
    consts[0, 129:385] = 1.0
    consts[0, 385] = EPS

    nf16 = nf.astype(np.float16)
    in_maps = []
    for c in range(NCORES):
        lo = slice(c * NLOC, (c + 1) * NLOC)
        el = ef[lo]                                        # [256,48,384]
        E = el.transpose(1, 0, 2).reshape(T, 384).astype(np.float16)  # k-major
        idx_k = np.ascontiguousarray(idx[lo].T).reshape(T)     # k-major values
        nfg = nf16[idx_k]                                  # [T,384] host gather
        comb = np.concatenate([E, nfg], axis=1)            # [T,768]
        edge = np.ascontiguousarray(
            comb.reshape(NCH, CH, 6, 128).transpose(0, 3, 2, 1))
        x0 = np.ascontiguousarray(
            nf[lo].reshape(NLOC, 3, 128).transpose(2, 1, 0))   # [128,3,256]
        in_maps.append(dict(
            edge=edge, wm=wm, wf=wf, lnpk=lnpk,
            consts=consts, x0=x0,
            mask=np.ascontiguousarray(mask[lo])[None, :]))
    return in_maps


def _unshard(results):
    out = np.empty((N, NF), np.float32)
    for c in range(NCORES):
        xfm = results[c]["out_x"]                          # [128,3,256]
        out[c * NLOC:(c + 1) * NLOC] = xfm.transpose(2, 1, 0).reshape(NLOC, NF)
    return out


def kernel(**inputs):
    nc = _get_nc()
    in_maps = _marshal(inputs)
    res = run_bass_kernel_spmd(nc, in_maps, list(range(NCORES)), trace=False)
    return _unshard(res.results)


# revision 18
# speedup vs baseline: 1.2037x; 1.0094x over previous
"""Trainium2 Bass kernel for nn_Decoder_60627758350737 (GNN message passing).

Sharding: node dim N=2048 split across 8 cores (256 nodes each). All MLP
weights replicated; no collectives.

The reference computes layer_edge_features once from the *initial* node
features, and neighbor_indices is a host-visible input — so the neighbor
gather nf0[idx] is done on the host during marshaling and shipped
feature-major alongside the edge features ([128, 6, 1024] fp16 per chunk:
kt 0-2 edges, kt 3-5 gathered nf0). The device applies [W0e | W0n] as a
768-deep contraction inside h0 — no on-device gather at all.

All message-MLP matmuls run in fp16 (full PE rate + fast weight load;
fp32-HIGH mode is ~1.3x slower per column and blocks FWL). Layout is
feature-major ([feature_part, token]); tokens are processed in 12 chunks
of 1024 (k-major: t = k*256 + n). Per chunk (paired 512-col matmuls so
each weight block loads once):
  h0   = gelu(W0en.T@[edge|nfg] + I.T@xw_b)   (xw_b = x@W0x + b0)
  h1   = gelu(W1.T@h0 + b1)
  hsum = sum of the 4 k-group slices of h1 (vector, fp16)
  agg += W2.T@hsum                            (PSUM accumulate across chunks)
then x1 = LN(x + agg/30 + 1.6*b2); x = mask*LN(x1 + dense(x1)) in fp32.
LN mean/var via ones-vector matmuls and rank-1 outer-product apply.
NOTE: agg's three 256-col regions share PSUM banks; only the first region
per bank may issue start=True (bank-wide pending-zero semantics).
"""
import numpy as np
import concourse.bass as bass
import concourse.bacc as bacc
import concourse.mybir as mybir
from concourse import tile
from concourse.bass_utils import run_bass_kernel_spmd
from contextlib import ExitStack

F32 = mybir.dt.float32
F32R = mybir.dt.float32r
F16 = mybir.dt.float16
AF = mybir.ActivationFunctionType

N, K, NF, L = 2048, 48, 384, 3
NCORES = 8
NLOC = N // NCORES            # 256
T = NLOC * K                  # 12288 tokens (k-major: t = k*256 + n)
CH = 1024                     # chunk (tokens)
NCH = T // CH                 # 12
SCALE = 30.0
EPS = 1e-5

# fp16 weight blob column offsets (per layer, [128, WMC])
O_W0X = 0
O_W0EN = 1152
O_W1 = 3456
O_W2 = 4608
O_DW0 = 5760
O_DW1 = 10368
WMC = 14976
# f32 bias blob columns (per layer, [128, WFC])
O_B0 = 0
O_B1 = 3
O_B2 = 6
O_DB0 = 9
O_DB1 = 21
WFC = 24

_NC_CACHE = {}


def _emit(act=None, layers=L):
    act = AF.Gelu if act is None else act
    nc = bacc.Bacc()
    edge_p = nc.declare_dram_parameter("edge", [NCH, 128, 6, CH], F16, isOutput=False)
    wm_p = nc.declare_dram_parameter("wm", [L, 128, WMC], F16, isOutput=False)
    wf_p = nc.declare_dram_parameter("wf", [L, 128, WFC], F32, isOutput=False)
    ln_p = nc.declare_dram_parameter("lnpk", [L, 1, 1536], F32R, isOutput=False)
    cst_p = nc.declare_dram_parameter("consts", [128, 386], F32R, isOutput=False)
    x0_p = nc.declare_dram_parameter("x0", [128, 3, NLOC], F32R, isOutput=False)
    mask_p = nc.declare_dram_parameter("mask", [1, NLOC], F32, isOutput=False)
    out_p = nc.declare_dram_parameter("out_x", [128, 3, NLOC], F32, isOutput=True)

    with tile.TileContext(nc) as tc, ExitStack() as ctx:
        wpool = ctx.enter_context(tc.tile_pool(name="w", bufs=2))
        epool = ctx.enter_context(tc.tile_pool(name="ep", bufs=2))
        xgpool = ctx.enter_context(tc.tile_pool(name="xgp", bufs=2))
        xg1 = ctx.enter_context(tc.tile_pool(name="xg1", bufs=1))
        hpool = ctx.enter_context(tc.tile_pool(name="hp", bufs=2))
        work1 = ctx.enter_context(tc.tile_pool(name="work1", bufs=1))
        xpool = ctx.enter_context(tc.tile_pool(name="xp", bufs=2))
        small = ctx.enter_context(tc.tile_pool(name="small", bufs=1))
        mm = ctx.enter_context(tc.tile_pool(name="mm", bufs=2, space="PSUM"))
        aggp = ctx.enter_context(tc.tile_pool(name="aggp", bufs=1, space="PSUM"))

        # --- one-time loads ---
        cst = small.tile([128, 386], F32R, tag="cst")
        nc.gpsimd.dma_start(cst[:], cst_p[:])
        maskt = small.tile([1, NLOC], F32, tag="maskt")
        nc.gpsimd.dma_start(maskt[:], mask_p[:])
        x = xpool.tile([128, 3, NLOC], F32R, tag="x")
        nc.gpsimd.dma_start(x[:], x0_p[:])

        ones_col = cst[:, 0:1]            # [128,1] 1/NF (stats lhsT -> means)
        ones_row = cst[0:1, 129:385]      # [1,256] ones
        eps_ap = cst[0:1, 385:386]        # [1,1] EPS

        et0 = epool.tile([128, 6, CH], F16, tag="et", name="et")
        nc.sync.dma_start(et0[:], edge_p[0])
        xb_t = [None]

        for l in range(layers):
            wm = wpool.tile([128, WMC], F16, tag="wm")
            nc.sync.dma_start(wm[:, 0:O_W1], wm_p[l][:, 0:O_W1])
            nc.gpsimd.dma_start(wm[:, O_W1:O_DW0], wm_p[l][:, O_W1:O_DW0])
            nc.gpsimd.dma_start(wm[:, O_DW0:WMC], wm_p[l][:, O_DW0:WMC])
            wf = wpool.tile([128, WFC], F32, tag="wf")
            nc.gpsimd.dma_start(wf[:], wf_p[l])
            lnw = wpool.tile([1, 1536], F32R, tag="lnw")
            nc.gpsimd.dma_start(lnw[:], ln_p[l])

            # --- xw_b = x @ W0x + b0  (fp16, feature-major), replicated x4 ---
            if xb_t[0] is None:
                xb = xpool.tile([128, 3, NLOC], F16, tag="xb")
                nc.vector.tensor_copy(xb[:], x[:].bitcast(F32))
            else:
                xb = xb_t[0]
            xwp = mm.tile([128, 3, 512], F32, tag="mm")
            for mt in range(3):
                for kt in range(3):
                    nc.tensor.matmul(
                        xwp[:, mt, 0:256],
                        wm[:, O_W0X + kt * 384 + mt * 128: O_W0X + kt * 384 + (mt + 1) * 128],
                        xb[:, kt, :],
                        start=(kt == 0), stop=(kt == 2))
            xw4 = work1.tile([128, 3, CH], F16, tag="xw4")
            for mt in range(3):
                nc.scalar.activation(xw4[:, mt, 0:256], xwp[:, mt, 0:256],
                                     AF.Identity,
                                     bias=wf[:, O_B0 + mt:O_B0 + mt + 1])
            for r in range(1, 4):
                nc.vector.tensor_copy(xw4[:, :, r * 256:(r + 1) * 256],
                                      xw4[:, :, 0:256])

            # b2s = b2 * (K/SCALE)
            b2s = small.tile([128, 3], F32, tag="b2s")
            nc.vector.tensor_scalar_mul(b2s[:], wf[:, O_B2:O_B2 + 3], K / SCALE)

            # --- k loop: 24 software-pipelined half-iterations of 512
            # tokens. Emission order per step i keeps the PE stream dense:
            # hp(i) MMs run while h0g(i-1) act / h1g(i-2) acts complete, so
            # the PSUM WAR chain never stalls the PE. agg for chunk c is
            # delayed two steps so its hsum input is long done.
            agg = aggp.tile([128, 768], F32, tag="agg")
            NH = 2 * NCH
            et_t = [None] * NCH
            h0g_t = [None] * NH
            h1g_t = [None] * NH
            sh_t = [None] * NH

            def emit_h0(i):
                cc, h = i // 2, i % 2
                if h == 0:
                    if l == 0 and cc == 0:
                        et_t[cc] = et0
                    else:
                        et_t[cc] = epool.tile([128, 6, CH], F16, tag="et",
                                              name="et")
                        nc.sync.dma_start(et_t[cc][:], edge_p[cc])
                et = et_t[cc]
                hp = mm.tile([128, 3, 512], F32, tag="mm", name="hp")
                for mt in range(3):
                    for kt in range(6):
                        off = O_W0EN + kt * 384 + mt * 128
                        nc.tensor.matmul(
                            hp[:, mt, :], wm[:, off:off + 128],
                            et[:, kt, h * 512:(h + 1) * 512],
                            start=(kt == 0), stop=(kt == 5))
                h0pre = hpool.tile([128, 3, 512], F16, tag=f"h0pre{i % 2}",
                                   name="h0pre")
                nc.vector.tensor_add(h0pre[:], hp[:],
                                     xw4[:, :, h * 512:(h + 1) * 512])
                h0g_t[i] = hpool.tile([128, 3, 512], F16, tag=f"h0g{i % 2}",
                                      name="h0g")
                nc.scalar.activation(h0g_t[i][:].rearrange("p a b -> p (a b)"),
                                     h0pre[:].rearrange("p a b -> p (a b)"),
                                     act)

            def emit_h1(i):
                h1p = mm.tile([128, 3, 512], F32, tag="mm", name="h1p")
                for mt in range(3):
                    for kt in range(3):
                        off = O_W1 + kt * 384 + mt * 128
                        nc.tensor.matmul(
                            h1p[:, mt, :], wm[:, off:off + 128],
                            h0g_t[i][:, kt, :],
                            start=(kt == 0), stop=(kt == 2))
                h1g = hpool.tile([128, 3, 512], F16, tag=f"h1g{i % 2}",
                                 name="h1g")
                for mt in range(3):
                    nc.scalar.activation(h1g[:, mt, :], h1p[:, mt, :], act,
                                         bias=wf[:, O_B1 + mt:O_B1 + mt + 1])
                sh_t[i] = xg1.tile([128, 3, 256], F16, tag=f"sh{i % 5}",
                                     name="sh")
                nc.vector.tensor_add(sh_t[i][:], h1g[:, :, 0:256],
                                     h1g[:, :, 256:512])

            def emit_agg(p):
                # p indexes a pair of chunks (8 k-groups pre-summed on DVE).
                # mt=1 shares a PSUM bank with mt=0: start=True pends the
                # whole 2KB zero region, so mt=1 must NOT issue start.
                ha = xg1.tile([128, 3, 256], F16, tag="ha", name="ha")
                nc.vector.tensor_add(ha[:], sh_t[4 * p][:], sh_t[4 * p + 1][:])
                hb = xg1.tile([128, 3, 256], F16, tag="hb", name="hb")
                nc.vector.tensor_add(hb[:], sh_t[4 * p + 2][:],
                                     sh_t[4 * p + 3][:])
                hs = xg1.tile([128, 3, 256], F16, tag="hs", name="hs")
                nc.vector.tensor_add(hs[:], ha[:], hb[:])
                for mt in range(3):
                    for kt in range(3):
                        off = O_W2 + kt * 384 + mt * 128
                        nc.tensor.matmul(
                            agg[:, mt * 256:(mt + 1) * 256],
                            wm[:, off:off + 128], hs[:, kt, :],
                            start=(p == 0 and kt == 0 and mt != 1),
                            stop=(p == NCH // 2 - 1 and kt == 2),
                            skip_group_check=True)

            for i in range(NH + 2):
                if i < NH:
                    emit_h0(i)
                if 1 <= i <= NH:
                    emit_h1(i - 1)
                if i >= 5 and (i - 5) % 4 == 0 and (i - 5) // 4 < NCH // 2:
                    emit_agg((i - 5) // 4)

            # --- x1_pre = x + agg + b2*K/SCALE (W2 is host-scaled by 1/30)
            dum = small.tile([1, 1], F32, tag="dum")
            nc.scalar.activation(dum[:], cst[0:1, 0:1].bitcast(F32), AF.Sqrt)
            x1p = work1.tile([128, 3, 256], F32R, tag="x1p")
            sq1 = work1.tile([128, 3, 256], F32R, tag="sq1")
            for mt in range(3):
                nc.vector.scalar_tensor_tensor(
                    x1p[:, mt, :], agg[:, mt * 256:(mt + 1) * 256],
                    b2s[:, mt:mt + 1], x[:, mt, :].bitcast(F32),
                    op0=mybir.AluOpType.add, op1=mybir.AluOpType.add)
                nc.vector.tensor_mul(sq1[:, mt, :], x1p[:, mt, :].bitcast(F32),
                                     x1p[:, mt, :].bitcast(F32))

            def layernorm(src, ln_i, masked, sq=None):
                """src: [128,3,256] F32R tile -> returns new [128,3,256] tile."""
                if sq is None:
                    sq = work1.tile([128, 3, 256], F32R, tag="sq")
                    nc.vector.tensor_mul(sq[:], src[:].bitcast(F32),
                                         src[:].bitcast(F32))
                st = mm.tile([128, 3, 512], F32, tag="mm")
                for kt in range(3):
                    nc.tensor.matmul(st[0:1, 0, 0:256], ones_col, src[:, kt, :],
                                     start=(kt == 0), stop=(kt == 2))
                for kt in range(3):
                    nc.tensor.matmul(st[0:1, 0, 256:512], ones_col, sq[:, kt, :],
                                     start=(kt == 0), stop=(kt == 2))
                # st[0,0:256]=mean, st[0,256:512]=E[x^2] (ones_col is 1/NF)
                m_ap = small.tile([1, 256], F32, tag="m_ap")
                nc.vector.tensor_copy(m_ap[:], st[0:1, 0, 0:256])
                m_ap = m_ap[:]
                msq = small.tile([1, 256], F32, tag="msq")
                nc.vector.tensor_mul(msq[:], m_ap, m_ap)
                var = small.tile([1, 256], F32, tag="var")
                nc.vector.tensor_sub(var[:], st[0:1, 0, 256:512], msq[:])
                sd = small.tile([1, 256], F32, tag="sd")
                nc.scalar.activation(sd[:], var[:], AF.Sqrt,
                                     bias=eps_ap.bitcast(F32))
                # rv rows (f32r): [0:256]=rstd(*mask), [256:512]=-m*rstd(*mask),
                # [512:768]= ones or mask
                rv = small.tile([1, 768], F32R, tag="rv")
                if masked:
                    rstd = small.tile([1, 256], F32, tag="rstd")
                    nc.vector.reciprocal_approx_fast(rstd[:], sd[:])
                    nc.vector.tensor_mul(rv[0:1, 0:256], rstd[:], maskt[:])
                    nmr = small.tile([1, 256], F32, tag="nmr")
                    nc.vector.scalar_tensor_tensor(
                        nmr[:], m_ap, -1.0, rstd[:],
                        op0=mybir.AluOpType.mult, op1=mybir.AluOpType.mult)
                    nc.vector.tensor_mul(rv[0:1, 256:512], nmr[:], maskt[:])

                    nc.vector.tensor_copy(rv[0:1, 512:768], maskt[:])
                else:
                    rstd = small.tile([1, 256], F32, tag="rstd")
                    nc.vector.reciprocal_approx_fast(rstd[:], sd[:])
                    nc.vector.tensor_copy(rv[0:1, 0:256], rstd[:])
                    nc.vector.scalar_tensor_tensor(
                        rv[0:1, 256:512], m_ap, -1.0, rstd[:],
                        op0=mybir.AluOpType.mult, op1=mybir.AluOpType.mult)
                    nc.vector.tensor_copy(rv[0:1, 512:768], ones_row.bitcast(F32))
                stp = mm.tile([128, 3, 512], F32, tag="mm")
                for mt in range(3):
                    woff = ln_i * 384 + mt * 128
                    nc.tensor.matmul(stp[:, mt, 0:256], lnw[0:1, woff:woff + 128],
                                     rv[0:1, 0:256], start=True, stop=True)
                    nc.tensor.matmul(stp[:, mt, 256:512],
                                     lnw[0:1, 768 + woff:768 + woff + 128],
                                     rv[0:1, 512:768], start=True, stop=False)
                    nc.tensor.matmul(stp[:, mt, 256:512], lnw[0:1, woff:woff + 128],
                                     rv[0:1, 256:512], start=False, stop=True)
                outt = xpool.tile([128, 3, 256], F32R, tag="lnout")
                out16 = xpool.tile([128, 3, 256], F16, tag="lnout16")
                for mt in range(3):
                    nc.vector.tensor_mul(outt[:, mt, :], src[:, mt, :].bitcast(F32),
                                         stp[:, mt, 0:256])
                    nc.vector.tensor_add(outt[:, mt, :], outt[:, mt, :].bitcast(F32),
                                         stp[:, mt, 256:512])
                    nc.vector.tensor_copy(out16[:, mt, :],
                                          outt[:, mt, :].bitcast(F32))
                return outt, out16

            x1, x1b = layernorm(x1p, 0, masked=False, sq=sq1)

            # --- dense MLP: d0 = gelu(x1 @ dw0 + db0); d1 = d0 @ dw1 + db1 ---
            d0g = work1.tile([128, 12, 256], F16, tag="d0g")
            for half in range(2):
                dp = mm.tile([128, 3, 512], F32, tag="mm")
                for m6 in range(6):
                    mt = half * 6 + m6
                    reg = dp[:, m6 // 2, (m6 % 2) * 256:(m6 % 2) * 256 + 256]
                    for kt in range(3):
                        nc.tensor.matmul(
                            reg,
                            wm[:, O_DW0 + kt * 1536 + mt * 128: O_DW0 + kt * 1536 + (mt + 1) * 128],
                            x1b[:, kt, :],
                            start=(kt == 0), stop=(kt == 2))
                    nc.scalar.activation(d0g[:, mt, :], reg, act,
                                         bias=wf[:, O_DB0 + mt:O_DB0 + mt + 1])
            d1p = mm.tile([128, 3, 512], F32, tag="mm")
            for kh in range(2):
                for mt in range(3):
                    for k6 in range(6):
                        kt = kh * 6 + k6
                        nc.tensor.matmul(
                            d1p[:, mt, 0:256],
                            wm[:, O_DW1 + kt * 384 + mt * 128: O_DW1 + kt * 384 + (mt + 1) * 128],
                            d0g[:, kt, :],
                            start=(kt == 0), stop=(kt == 11),
                            skip_group_check=True)
            dum2 = small.tile([1, 1], F32, tag="dum2")
            nc.scalar.activation(dum2[:], cst[0:1, 0:1].bitcast(F32), AF.Sqrt)
            x2p = work1.tile([128, 3, 256], F32R, tag="x2p")
            sq2 = work1.tile([128, 3, 256], F32R, tag="sq2")
            for mt in range(3):
                nc.vector.scalar_tensor_tensor(
                    x2p[:, mt, :], d1p[:, mt, 0:256],
                    wf[:, O_DB1 + mt:O_DB1 + mt + 1], x1[:, mt, :].bitcast(F32),
                    op0=mybir.AluOpType.add, op1=mybir.AluOpType.add)
                nc.vector.tensor_mul(sq2[:, mt, :], x2p[:, mt, :].bitcast(F32),
                                     x2p[:, mt, :].bitcast(F32))

            x, xb_next = layernorm(x2p, 1, masked=True, sq=sq2)
            xb_t[0] = xb_next

        nc.sync.dma_start(out_p[:], x[:].bitcast(F32))

    nc.finalize()
    return nc


def _get_nc():
    if "nc" not in _NC_CACHE:
        _NC_CACHE["nc"] = _emit()
    return _NC_CACHE["nc"]


def _fm(w):
    """[in, out] fp32 -> [128, n_kt*out] (feature-major lhsT blob columns)."""
    i, o = w.shape
    return np.ascontiguousarray(
        w.reshape(i // 128, 128, o).transpose(1, 0, 2).reshape(128, -1))


def _marshal(inputs):
    nf = np.asarray(inputs["node_features"], np.float32)
    ef = np.asarray(inputs["edge_features"], np.float32)
    idx = np.asarray(inputs["neighbor_indices"])
    mask = np.asarray(inputs["mask"], np.float32)

    # replicated tensors
    wm = np.empty((L, 128, WMC), np.float16)
    wf = np.empty((L, 128, WFC), np.float32)
    lnpk = np.empty((L, 1, 1536), np.float32)
    for l in range(L):
        w0 = np.asarray(inputs["msg_w0"], np.float32)[l]
        cols = [
            _fm(w0[0:384]), _fm(w0[384:768]), _fm(w0[1152:1536]),
            _fm(np.asarray(inputs["msg_w1"], np.float32)[l]),
            _fm(np.asarray(inputs["msg_w2"], np.float32)[l] / SCALE),
            _fm(np.asarray(inputs["dense_w0"], np.float32)[l]),
            _fm(np.asarray(inputs["dense_w1"], np.float32)[l]),
        ]
        wm[l] = np.concatenate(cols, axis=1).astype(np.float16)
        fcols = [
            np.asarray(inputs["msg_b0"], np.float32)[l].reshape(3, 128).T,
            np.asarray(inputs["msg_b1"], np.float32)[l].reshape(3, 128).T,
            np.asarray(inputs["msg_b2"], np.float32)[l].reshape(3, 128).T,
            np.asarray(inputs["dense_b0"], np.float32)[l].reshape(12, 128).T,
            np.asarray(inputs["dense_b1"], np.float32)[l].reshape(3, 128).T,
        ]
        wf[l] = np.concatenate(fcols, axis=1)
        lnpk[l, 0] = np.concatenate([
            np.asarray(inputs["ln1_w"], np.float32)[l],
            np.asarray(inputs["ln2_w"], np.float32)[l],
            np.asarray(inputs["ln1_b"], np.float32)[l],
            np.asarray(inputs["ln2_b"], np.float32)[l]])
    consts = np.zeros((128, 386), np.float32)
    consts[:, 0] = 1.0 / NF
    consts[:, 1:129] = np.eye(128, dtype=np.float32)
    consts[0, 129:385] = 1.0
    consts[0, 385] = EPS

    nf16 = nf.astype(np.float16)
    in_maps = []
    for c in range(NCORES):
        lo = slice(c * NLOC, (c + 1) * NLOC)
        el = ef[lo]                                        # [256,48,384]
        E = el.transpose(1, 0, 2).reshape(T, 384).astype(np.float16)  # k-major
        idx_k = np.ascontiguousarray(idx[lo].T).reshape(T)     # k-major values
        nfg = nf16[idx_k]                                  # [T,384] host gather
        comb = np.concatenate([E, nfg], axis=1)            # [T,768]
        edge = np.ascontiguousarray(
            comb.reshape(NCH, CH, 6, 128).transpose(0, 3, 2, 1))
        x0 = np.ascontiguousarray(
            nf[lo].reshape(NLOC, 3, 128).transpose(2, 1, 0))   # [128,3,256]
        in_maps.append(dict(
            edge=edge, wm=wm, wf=wf, lnpk=lnpk,
            consts=consts, x0=x0,
            mask=np.ascontiguousarray(mask[lo])[None, :]))
    return in_maps


def _unshard(results):
    out = np.empty((N, NF), np.float32)
    for c in range(NCORES):
        xfm = results[c]["out_x"]                          # [128,3,256]
        out[c * NLOC:(c + 1) * NLOC] = xfm.transpose(2, 1, 0).reshape(NLOC, NF)
    return out


def kernel(**inputs):
    nc = _get_nc()
    in_maps = _marshal(inputs)
    res = run_bass_kernel_spmd(nc, in_maps, list(range(NCORES)), trace=False)
    return _unshard(res.results)
